# revision 16
# baseline (speedup 1.0000x reference)
"""Trainium2 Bass kernel for CosyVoice3 DiT attention (B=2, S=2048, H=16, hd=64, D=1024).

Sharding: tensor parallelism over heads — 2 heads per core on 8 cores.
Each core computes QKV projections for its head slice, RoPE, full attention
for its 2 heads, then its heads' contribution to the output projection
(row-parallel). The host gather sums the 8 partial outputs (the standard
row-parallel TP reduction) and adds the output bias.

Layout trick: everything is computed transposed ([dim, tokens]) so the
attention matmuls need no on-chip transposes of the big S x S matrices:
  scoresT[k,q] = K @ Q^T    (lhsT = K^T slice, rhs = Q^T slice)
  outT[d,q]    = V_aug^T @ expT  with V_aug = [V | ones] giving the softmax
                 denominator for free in row 64.
Softmax skips max-subtraction (scores are O(10) for this model family, and
exp is computed in fp32 which is safe up to ~88).

The emission order interleaves batch-0 attention with the tail of the
QKV-projection phase so the PE never idles long enough for the HAM clock
gate to re-throttle it to 1.2 GHz.
"""
import sys
sys.path.insert(0, "/opt/trn_rl_repo")
from contextlib import ExitStack
import numpy as np

# NTFF profile hook shim: this image's antenv lacks axon_hooks, which
# bass_utils imports unconditionally when trace=True (and the boot-time
# installer degrades silently without it). Recreate the module and install
# the ctypes-based hook so neuron-profile traces work.
import types as _types
try:
    import antenv as _antenv
    if "antenv.axon_hooks" not in sys.modules:
        _hooks = _types.ModuleType("antenv.axon_hooks")
        _hook_box = [None]
        _hooks.set_axon_ntff_profile_hook = lambda h: _hook_box.__setitem__(0, h)
        _hooks.get_axon_ntff_profile_hook = lambda: _hook_box[0]
        sys.modules["antenv.axon_hooks"] = _hooks
        _antenv.axon_hooks = _hooks
        try:
            from trn_agent_boot.trn_boot import _ntff_profile_via_ctypes
            _hooks.set_axon_ntff_profile_hook(
                _ntff_profile_via_ctypes("/opt/axon/libaxon_pjrt.so"))
        except Exception:
            pass
except Exception:
    pass

import concourse.bass as bass
import concourse.mybir as mybir
from concourse import bacc
from concourse.tile import TileContext
from concourse.bass_interp import get_hw_module
from concourse import bass_utils
from concourse.masks import make_identity
bass_utils.upload_artifacts = lambda tmpdir: str(tmpdir)  # no S3 in container

# ── constants (hardcoded per problem spec) ────────────────────────────────
B, S, D, H, HD = 2, 2048, 1024, 16, 64
T = B * S                 # 4096 tokens
NC = 8                    # cores
HPC = H // NC             # 2 heads per core
CW = HPC * HD             # 128 rows/cols per core
SCALE = 1.0 / np.sqrt(HD)
F32 = mybir.dt.float32
F32R = mybir.dt.float32r
BF16 = mybir.dt.bfloat16
AF = mybir.ActivationFunctionType

_CACHE = {}


def _build(use_mask: bool):
    nc = bacc.Bacc("TRN2", target_bir_lowering=False, debug=False, num_devices=NC)

    # inputs (per-core slices supplied by host)
    x_d = nc.dram_tensor("x", [T, D], F32, kind="ExternalInput")
    wq_d = nc.dram_tensor("wq", [D, CW], F32R, kind="ExternalInput")
    wk_d = nc.dram_tensor("wk", [D, CW], F32R, kind="ExternalInput")
    wv_d = nc.dram_tensor("wv", [D, CW], F32R, kind="ExternalInput")
    # wo: the CW rows of Wo owned by this core's heads -> [CW, D]
    wo_d = nc.dram_tensor("wo", [CW, D], F32R, kind="ExternalInput")
    bq_d = nc.dram_tensor("bq", [CW, 1], F32, kind="ExternalInput")
    bk_d = nc.dram_tensor("bk", [CW, 1], F32, kind="ExternalInput")
    bv_d = nc.dram_tensor("bv", [CW, 1], F32, kind="ExternalInput")
    cos_d = nc.dram_tensor("cost", [CW, T], F32, kind="ExternalInput")
    sin_d = nc.dram_tensor("sint", [CW, T], F32, kind="ExternalInput")   # sign-folded
    psw_d = nc.dram_tensor("pswap", [128, 128], F32R, kind="ExternalInput")
    if use_mask:
        mt_d = nc.dram_tensor("maskt", [S, S], F32, kind="ExternalInput")

    # partial output, transposed: ypT[n, t] = sum over this core's head dims
    ypT_d = nc.dram_tensor("ypT", [D, T], F32, kind="ExternalOutput")

    NCHUNK = 8            # token chunks of 512 for projections
    CH = T // NCHUNK      # 512
    KT = S // 128         # 16 k-tiles per batch
    QW = 512              # q chunk width
    QC = S // QW          # 4 q chunks per batch

    with TileContext(nc) as tc:
        with tc.tile_pool(name="persist", bufs=1) as persist, \
             tc.tile_pool(name="wpool", bufs=1) as wpool, \
             tc.tile_pool(name="xload", bufs=6) as xload, \
             tc.tile_pool(name="xtp", bufs=10) as xtpool, \
             tc.tile_pool(name="chunks", bufs=2) as chunks, \
             tc.tile_pool(name="expp", bufs=4) as expp, \
             tc.tile_pool(name="outp", bufs=3) as outp:

            # ── persistent tiles ────────────────────────────────────────
            ident = persist.tile([128, 128], F32, name="ident")
            make_identity(nc, ident)
            psw = persist.tile([128, 128], F32R, name="psw")
            nc.sync.dma_start(out=psw, in_=psw_d[:, :])
            wq = wpool.tile([128, D // 128, CW], F32R, name="wq_sb")
            wk = wpool.tile([128, D // 128, CW], F32R, name="wk_sb")
            wv = wpool.tile([128, D // 128, CW], F32R, name="wv_sb")
            for wt, wdr in ((wq, wq_d), (wk, wk_d), (wv, wv_d)):
                nc.sync.dma_start(out=wt, in_=wdr.ap().rearrange("(kc p) m -> p kc m", p=128))
            # wo rows for this core: [CW, D] -> lhsT chunks [CW, 128] per out-col group
            wo = wpool.tile([CW, D // 128, 128], F32R, name="wo_sb")
            nc.sync.dma_start(out=wo, in_=wo_d.ap().rearrange("p (mc m) -> p mc m", m=128))
            bq = wpool.tile([CW, 1], F32, name="bq_sb")
            bk = wpool.tile([CW, 1], F32, name="bk_sb")
            bv0 = wpool.tile([HD, 1], F32, name="bv0_sb")
            bv1 = wpool.tile([HD, 1], F32, name="bv1_sb")
            nc.sync.dma_start(out=bq, in_=bq_d[:, :])
            nc.sync.dma_start(out=bk, in_=bk_d[:, :])
            nc.sync.dma_start(out=bv0, in_=bv_d[0:HD, :])
            nc.sync.dma_start(out=bv1, in_=bv_d[HD:CW, :])

            qtr = persist.tile([128, T], BF16, name="qtr")    # rope'd Q^T
            ktr = persist.tile([128, T], BF16, name="ktr")    # rope'd K^T
            aoT = persist.tile([128, T], F32R, name="aoT")    # normalized attn out^T
            # V natural per k-tile: [128 tok, 2*(64+1)] with ones cols
            vnat = [persist.tile([128, 2 * (HD + 1)], BF16, name=f"vnat{i}")
                    for i in range(T // 128)]

            # ── phase 1: per token-chunk: transpose x, QKV proj, rope ───
            def emit_chunk(n):
                tcol = n * CH
                xts = [xtpool.tile([128, CH], F32R, name=f"xt{n}_{dc}", tag="xt")
                       for dc in range(D // 128)]
                xns = []
                for tt in range(CH // 128):
                    xn = xload.tile([128, D], F32, name=f"xn{n}_{tt}", tag="xn")
                    nc.sync.dma_start(out=xn, in_=x_d[tcol + 128 * tt: tcol + 128 * (tt + 1), :])
                    xns.append(xn)
                for dc in range(D // 128):
                    # pack the 4 token-block transposes of one d-block into one
                    # psum bank, one copy out
                    xp = pools["tp"].tile([128, CH], F32, name=f"xp{n}{dc}", tag="tp")
                    for tt in range(CH // 128):
                        nc.tensor.transpose(xp[:, 128 * tt:128 * (tt + 1)],
                                            xns[tt][:, 128 * dc:128 * (dc + 1)], ident)
                    nc.any.tensor_copy(xts[dc][:, :], xp[:, :])

                cos_c = chunks.tile([128, CH], F32, name=f"cos{n}", tag="cosc")
                sin_c = chunks.tile([128, CH], F32, name=f"sin{n}", tag="sinc")
                nc.sync.dma_start(out=cos_c, in_=cos_d[:, tcol:tcol + CH])
                nc.sync.dma_start(out=sin_c, in_=sin_d[:, tcol:tcol + CH])

                for name, wt, dst in (("q", wq, qtr), ("k", wk, ktr), ("v", wv, None)):
                    pp = pools["proj"].tile([128, CH], F32, name=f"{name}ps{n}", tag="proj")
                    for dc in range(D // 128):
                        nc.tensor.matmul(pp[:, :], wt[:, dc, :], xts[dc][:, :],
                                         start=(dc == 0), stop=(dc == D // 128 - 1))
                    if name == "v":
                        # per-head natural V via PE transpose; ones row becomes
                        # the denominator column after transpose
                        for h in range(HPC):
                            vth = chunks.tile([HD + 1, CH], F32, name=f"vt{n}{h}", tag="vth")
                            nc.scalar.activation(vth[0:HD, :], pp[HD * h:HD * (h + 1), :],
                                                 AF.Identity, bias=(bv0 if h == 0 else bv1))
                            nc.vector.memset(vth[HD:HD + 1, :], 1.0)
                            for ktl in range(CH // 128):
                                vp = pools["tp"].tile([128, HD + 1], F32, name=f"vp{n}{h}{ktl}", tag="tp")
                                nc.tensor.transpose(vp[:, :], vth[:, 128 * ktl:128 * (ktl + 1)],
                                                    ident[0:HD + 1, 0:HD + 1])
                                kt_glob = (tcol + 128 * ktl) // 128
                                nc.any.tensor_copy(
                                    vnat[kt_glob][:, 65 * h:65 * h + HD + 1], vp[:, :])
                    else:
                        # bias + rope: dst_chunk = (p+b)*cos + swap(p+b)*sin_signed
                        qb = chunks.tile([128, CH], F32R, name=f"{name}b{n}", tag="qb")
                        nc.scalar.activation(qb[:, :], pp[:, :], AF.Identity,
                                             bias=(bq if name == "q" else bk))
                        sw = pools["tp"].tile([128, CH], F32, name=f"{name}sw{n}", tag="tp")
                        for j in range(CH // 512):
                            nc.tensor.matmul(sw[:, 512 * j:512 * (j + 1)], psw,
                                             qb[:, 512 * j:512 * (j + 1)],
                                             start=True, stop=True)
                        t1 = chunks.tile([128, CH], F32, name=f"{name}t1{n}", tag="t1")
                        t2 = chunks.tile([128, CH], F32, name=f"{name}t2{n}", tag="t2")
                        nc.vector.tensor_mul(t1[:, :], qb[:, :], cos_c[:, :])
                        nc.vector.tensor_mul(t2[:, :], sw[:, :], sin_c[:, :])
                        nc.vector.tensor_add(dst[:, tcol:tcol + CH], t1[:, :], t2[:, :])

            # ── phase 3: attention for one (batch, head, q-chunk) ───────
            def emit_att(b, h, qc):
                toff = b * S
                po = HD * h
                qcols = slice(toff + QW * qc, toff + QW * (qc + 1))
                ot = pools["ot"].tile([HD + 1, QW], F32, name=f"ot{b}{h}{qc}", tag="ot")

                def emit_scores(kt):
                    krows = slice(toff + 128 * kt, toff + 128 * (kt + 1))
                    sc = pools["sc"].tile([128, QW], F32, name=f"sc{b}{h}{qc}{kt}", tag="sc")
                    nc.tensor.matmul(sc[:, :], ktr[po:po + HD, krows],
                                     qtr[po:po + HD, qcols], start=True, stop=True)
                    if use_mask:
                        mtile = expp.tile([128, QW], F32, name=f"mt{b}{h}{qc}{kt}", tag="mt")
                        nc.sync.dma_start(
                            out=mtile,
                            in_=mt_d[128 * kt:128 * (kt + 1), QW * qc:QW * (qc + 1)])
                        nc.vector.tensor_scalar_mul(sc[:, :], sc[:, :], SCALE)
                        nc.vector.tensor_add(sc[:, :], sc[:, :], mtile[:, :])
                    ex = expp.tile([128, QW], BF16, name=f"ex{b}{h}{qc}{kt}", tag="ex")
                    nc.scalar.activation(ex[:, :], sc[:, :], AF.Exp,
                                         scale=(1.0 if use_mask else SCALE))
                    return ex

                def emit_av(kt, ex):
                    kt_glob = (toff + 128 * kt) // 128
                    nc.tensor.matmul(ot[:, :], vnat[kt_glob][:, 65 * h:65 * h + HD + 1],
                                     ex[:, :], start=(kt == 0), stop=(kt == KT - 1))

                # software-pipelined: scores(kt+1) issues before AV(kt) so the
                # in-order PE never waits on the exp between them
                exs = emit_scores(0)
                for kt in range(1, KT):
                    ex_next = emit_scores(kt)
                    emit_av(kt - 1, exs)
                    exs = ex_next
                emit_av(KT - 1, exs)
                # normalize: rows 0..63 divided by row 64
                rec = outp.tile([1, QW], F32, name=f"rec{b}{h}{qc}", tag="rec")
                nc.vector.reciprocal(rec[:, :], ot[HD:HD + 1, :])
                bcast = outp.tile([HD, QW], F32, name=f"bc{b}{h}{qc}", tag="bc")
                nc.gpsimd.partition_broadcast(bcast[:, :], rec[:, :])
                nc.vector.tensor_mul(aoT[po:po + HD, qcols], ot[0:HD, :], bcast[:, :])

            # ── phase 4: partial output projection for one (batch,qchunk)
            def emit_oproj(b, qc):
                toff = b * S
                qcols = slice(toff + QW * qc, toff + QW * (qc + 1))
                for mo in range(D // 128):
                    yp = pools["proj"].tile([128, QW], F32, name=f"yp{b}{qc}{mo}", tag="proj")
                    nc.tensor.matmul(yp[:, :], wo[:, mo, :], aoT[:, qcols],
                                     start=True, stop=True)
                    yo = outp.tile([128, QW], F32, name=f"yo{b}{qc}{mo}", tag="yo")
                    nc.any.tensor_copy(yo[:, :], yp[:, :])
                    nc.sync.dma_start(out=ypT_d[128 * mo:128 * (mo + 1), qcols], in_=yo)

            # ── emission: sequential phases, phase-scoped psum pools ────
            ps1 = ExitStack()
            ps_tp = ps1.enter_context(tc.tile_pool(name="ps_tp", bufs=4, space="PSUM"))
            ps_proj = ps1.enter_context(tc.tile_pool(name="ps_proj", bufs=3, space="PSUM"))
            pools = {"tp": ps_tp, "proj": ps_proj}
            for n in range(NCHUNK):
                emit_chunk(n)
            ps1.close()
            ps3 = ExitStack()
            pools["sc"] = ps3.enter_context(tc.tile_pool(name="ps_sc", bufs=4, space="PSUM"))
            pools["ot"] = ps3.enter_context(tc.tile_pool(name="ps_ot", bufs=2, space="PSUM"))
            pools["proj"] = ps3.enter_context(tc.tile_pool(name="ps_y", bufs=2, space="PSUM"))
            for b in range(B):
                for h in range(HPC):
                    for qc in range(QC):
                        emit_att(b, h, qc)
                for qc in range(QC):
                    emit_oproj(b, qc)
            ps3.close()

    nc.compile()
    nc.m = get_hw_module(nc.m)
    return nc


def _get_nc(use_mask: bool):
    key = ("nc", use_mask)
    if key not in _CACHE:
        _CACHE[key] = _build(use_mask)
    return _CACHE[key]


def kernel(x, rope, mask, Wq, bq, Wk, bk, Wv, bv, Wo, bo, _trace=False):
    x = np.ascontiguousarray(np.asarray(x, dtype=np.float32))
    rope = np.asarray(rope, dtype=np.float32)
    mask = np.asarray(mask, dtype=np.float32)
    use_mask = bool(np.any(mask))

    x2d = x.reshape(T, D)
    cos = rope[0, 0, :, 0, :]                      # [S, 64]
    sin = rope[1, 0, :, 0, :]
    sgn = np.where(np.arange(HD) % 2 == 0, -1.0, 1.0).astype(np.float32)[:, None]
    cosT = np.ascontiguousarray(np.tile(cos.T, (HPC, B)))          # [128, T]
    sinT = np.ascontiguousarray(np.tile(sin.T * sgn, (HPC, B)))    # [128, T]
    psw = np.zeros((128, 128), dtype=np.float32)
    idx = np.arange(128)
    psw[idx ^ 1, idx] = 1.0

    nc = _get_nc(use_mask)
    in_maps = []
    for c in range(NC):
        cs = slice(CW * c, CW * (c + 1))
        m = dict(
            x=x2d,
            wq=np.ascontiguousarray(Wq[:, cs]), bq=np.ascontiguousarray(bq[cs]).reshape(CW, 1),
            wk=np.ascontiguousarray(Wk[:, cs]), bk=np.ascontiguousarray(bk[cs]).reshape(CW, 1),
            wv=np.ascontiguousarray(Wv[:, cs]), bv=np.ascontiguousarray(bv[cs]).reshape(CW, 1),
            wo=np.ascontiguousarray(Wo[cs, :]),
            cost=cosT, sint=sinT, pswap=psw,
        )
        if use_mask:
            m["maskt"] = np.ascontiguousarray(mask[0, 0].T)
        in_maps.append({k: np.asarray(v, dtype=np.float32) for k, v in m.items()})

    res = bass_utils.run_bass_kernel_spmd(
        nc, in_maps, core_ids=list(range(NC)), trace=_trace)
    # row-parallel unshard: sum the per-core partial projections, add bias
    ypT = res.results[0]["ypT"].astype(np.float32)
    for c in range(1, NC):
        ypT = ypT + res.results[c]["ypT"]
    out = (ypT.T + np.asarray(bo, dtype=np.float32)).reshape(B, S, D).astype(np.float32)
    out = np.ascontiguousarray(out)
    if _trace:
        return out, res
    return out


# revision 17
# speedup vs baseline: 1.0961x; 1.0961x over previous
"""Trainium2 Bass kernel for CosyVoice3 DiT attention (B=2, S=2048, H=16, hd=64, D=1024).

Sharding: tensor parallelism over heads — 2 heads per core on 8 cores.
Each core computes QKV projections for its head slice, RoPE, full attention
for its 2 heads, then its heads' contribution to the output projection
(row-parallel). The host gather sums the 8 partial outputs (the standard
row-parallel TP reduction) and adds the output bias.

Layout trick: everything is computed transposed ([dim, tokens]) so the
attention matmuls need no on-chip transposes of the big S x S matrices:
  scoresT[k,q] = K @ Q^T    (lhsT = K^T slice, rhs = Q^T slice)
  outT[d,q]    = V_aug^T @ expT  with V_aug = [V | ones] giving the softmax
                 denominator for free in row 64.
Softmax skips max-subtraction (scores are O(10) for this model family, and
exp is computed in fp32 which is safe up to ~88).

The emission order interleaves batch-0 attention with the tail of the
QKV-projection phase so the PE never idles long enough for the HAM clock
gate to re-throttle it to 1.2 GHz.
"""
import sys
sys.path.insert(0, "/opt/trn_rl_repo")
from contextlib import ExitStack
import numpy as np

# NTFF profile hook shim: this image's antenv lacks axon_hooks, which
# bass_utils imports unconditionally when trace=True (and the boot-time
# installer degrades silently without it). Recreate the module and install
# the ctypes-based hook so neuron-profile traces work.
import types as _types
try:
    import antenv as _antenv
    if "antenv.axon_hooks" not in sys.modules:
        _hooks = _types.ModuleType("antenv.axon_hooks")
        _hook_box = [None]
        _hooks.set_axon_ntff_profile_hook = lambda h: _hook_box.__setitem__(0, h)
        _hooks.get_axon_ntff_profile_hook = lambda: _hook_box[0]
        sys.modules["antenv.axon_hooks"] = _hooks
        _antenv.axon_hooks = _hooks
        try:
            from trn_agent_boot.trn_boot import _ntff_profile_via_ctypes
            _hooks.set_axon_ntff_profile_hook(
                _ntff_profile_via_ctypes("/opt/axon/libaxon_pjrt.so"))
        except Exception:
            pass
except Exception:
    pass

import concourse.bass as bass
import concourse.mybir as mybir
from concourse import bacc
from concourse.tile import TileContext
from concourse.bass_interp import get_hw_module
from concourse import bass_utils
from concourse.masks import make_identity
bass_utils.upload_artifacts = lambda tmpdir: str(tmpdir)  # no S3 in container

# ── constants (hardcoded per problem spec) ────────────────────────────────
B, S, D, H, HD = 2, 2048, 1024, 16, 64
T = B * S                 # 4096 tokens
NC = 8                    # cores
HPC = H // NC             # 2 heads per core
CW = HPC * HD             # 128 rows/cols per core
SCALE = 1.0 / np.sqrt(HD)
F32 = mybir.dt.float32
F32R = mybir.dt.float32r
BF16 = mybir.dt.bfloat16
AF = mybir.ActivationFunctionType

_CACHE = {}


def _build(use_mask: bool):
    nc = bacc.Bacc("TRN2", target_bir_lowering=False, debug=False, num_devices=NC)

    # inputs (per-core slices supplied by host)
    x_d = nc.dram_tensor("x", [T, D], F32, kind="ExternalInput")
    wq_d = nc.dram_tensor("wq", [D, CW], F32R, kind="ExternalInput")
    wk_d = nc.dram_tensor("wk", [D, CW], F32R, kind="ExternalInput")
    wv_d = nc.dram_tensor("wv", [D, CW], F32R, kind="ExternalInput")
    # wo: the CW rows of Wo owned by this core's heads -> [CW, D]
    wo_d = nc.dram_tensor("wo", [CW, D], F32R, kind="ExternalInput")
    bq_d = nc.dram_tensor("bq", [CW, 1], F32, kind="ExternalInput")
    bk_d = nc.dram_tensor("bk", [CW, 1], F32, kind="ExternalInput")
    bv_d = nc.dram_tensor("bv", [CW, 1], F32, kind="ExternalInput")
    cos_d = nc.dram_tensor("cost", [CW, T], F32, kind="ExternalInput")
    sin_d = nc.dram_tensor("sint", [CW, T], F32, kind="ExternalInput")   # sign-folded
    psw_d = nc.dram_tensor("pswap", [128, 128], F32R, kind="ExternalInput")
    if use_mask:
        mt_d = nc.dram_tensor("maskt", [S, S], F32, kind="ExternalInput")

    # partial output, transposed: ypT[n, t] = sum over this core's head dims
    ypT_d = nc.dram_tensor("ypT", [D, T], F32, kind="ExternalOutput")

    NCHUNK = 8            # token chunks of 512 for projections
    CH = T // NCHUNK      # 512
    KT = S // 128         # 16 k-tiles per batch
    QW = 512              # q chunk width
    QC = S // QW          # 4 q chunks per batch

    with TileContext(nc) as tc:
        with tc.tile_pool(name="persist", bufs=1) as persist, \
             tc.tile_pool(name="wpool", bufs=1) as wpool, \
             tc.tile_pool(name="xload", bufs=6) as xload, \
             tc.tile_pool(name="xtp", bufs=10) as xtpool, \
             tc.tile_pool(name="chunks", bufs=2) as chunks, \
             tc.tile_pool(name="expp", bufs=4) as expp, \
             tc.tile_pool(name="outp", bufs=3) as outp:

            # ── persistent tiles ────────────────────────────────────────
            ident = persist.tile([128, 128], F32, name="ident")
            make_identity(nc, ident)
            psw = persist.tile([128, 128], F32R, name="psw")
            nc.sync.dma_start(out=psw, in_=psw_d[:, :])
            wq = wpool.tile([128, D // 128, CW], F32R, name="wq_sb")
            wk = wpool.tile([128, D // 128, CW], F32R, name="wk_sb")
            wv = wpool.tile([128, D // 128, CW], F32R, name="wv_sb")
            for wt, wdr in ((wq, wq_d), (wk, wk_d), (wv, wv_d)):
                nc.sync.dma_start(out=wt, in_=wdr.ap().rearrange("(kc p) m -> p kc m", p=128))
            # wo rows for this core: [CW, D] -> lhsT chunks [CW, 128] per out-col group
            wo = wpool.tile([CW, D // 128, 128], F32R, name="wo_sb")
            nc.sync.dma_start(out=wo, in_=wo_d.ap().rearrange("p (mc m) -> p mc m", m=128))
            bq = wpool.tile([CW, 1], F32, name="bq_sb")
            bk = wpool.tile([CW, 1], F32, name="bk_sb")
            bv0 = wpool.tile([HD, 1], F32, name="bv0_sb")
            bv1 = wpool.tile([HD, 1], F32, name="bv1_sb")
            nc.sync.dma_start(out=bq, in_=bq_d[:, :])
            nc.sync.dma_start(out=bk, in_=bk_d[:, :])
            nc.sync.dma_start(out=bv0, in_=bv_d[0:HD, :])
            nc.sync.dma_start(out=bv1, in_=bv_d[HD:CW, :])

            qtr = persist.tile([128, T], BF16, name="qtr")    # rope'd Q^T
            ktr = persist.tile([128, T], BF16, name="ktr")    # rope'd K^T
            aoT = persist.tile([128, T], F32R, name="aoT")    # normalized attn out^T
            # V natural per k-tile: [128 tok, 2*(64+1)] with ones cols
            vnat = [persist.tile([128, 2 * (HD + 1)], BF16, name=f"vnat{i}")
                    for i in range(T // 128)]

            # ── phase 1: per token-chunk: transpose x, QKV proj, rope ───
            def emit_chunk(n):
                tcol = n * CH
                xts = [xtpool.tile([128, CH], F32R, name=f"xt{n}_{dc}", tag="xt")
                       for dc in range(D // 128)]
                xns = []
                for tt in range(CH // 128):
                    xn = xload.tile([128, D], F32, name=f"xn{n}_{tt}", tag="xn")
                    nc.sync.dma_start(out=xn, in_=x_d[tcol + 128 * tt: tcol + 128 * (tt + 1), :])
                    xns.append(xn)
                for dc in range(D // 128):
                    # pack the 4 token-block transposes of one d-block into one
                    # psum bank, one copy out
                    xp = pools["tp"].tile([128, CH], F32, name=f"xp{n}{dc}", tag="tp")
                    for tt in range(CH // 128):
                        nc.tensor.transpose(xp[:, 128 * tt:128 * (tt + 1)],
                                            xns[tt][:, 128 * dc:128 * (dc + 1)], ident)
                    nc.any.tensor_copy(xts[dc][:, :], xp[:, :])

                cos_c = chunks.tile([128, CH], F32, name=f"cos{n}", tag="cosc")
                sin_c = chunks.tile([128, CH], F32, name=f"sin{n}", tag="sinc")
                nc.sync.dma_start(out=cos_c, in_=cos_d[:, tcol:tcol + CH])
                nc.sync.dma_start(out=sin_c, in_=sin_d[:, tcol:tcol + CH])

                for name, wt, dst in (("q", wq, qtr), ("k", wk, ktr), ("v", wv, None)):
                    pp = pools["proj"].tile([128, CH], F32, name=f"{name}ps{n}", tag="proj")
                    for dc in range(D // 128):
                        nc.tensor.matmul(pp[:, :], wt[:, dc, :], xts[dc][:, :],
                                         start=(dc == 0), stop=(dc == D // 128 - 1))
                    if name == "v":
                        # per-head natural V via PE transpose; ones row becomes
                        # the denominator column after transpose
                        for h in range(HPC):
                            vth = chunks.tile([HD + 1, CH], F32, name=f"vt{n}{h}", tag="vth")
                            nc.scalar.activation(vth[0:HD, :], pp[HD * h:HD * (h + 1), :],
                                                 AF.Identity, bias=(bv0 if h == 0 else bv1))
                            nc.vector.memset(vth[HD:HD + 1, :], 1.0)
                            for ktl in range(CH // 128):
                                vp = pools["tp"].tile([128, HD + 1], F32, name=f"vp{n}{h}{ktl}", tag="tp")
                                nc.tensor.transpose(vp[:, :], vth[:, 128 * ktl:128 * (ktl + 1)],
                                                    ident[0:HD + 1, 0:HD + 1])
                                kt_glob = (tcol + 128 * ktl) // 128
                                nc.any.tensor_copy(
                                    vnat[kt_glob][:, 65 * h:65 * h + HD + 1], vp[:, :])
                    else:
                        # bias + rope: dst_chunk = (p+b)*cos + swap(p+b)*sin_signed
                        qb = chunks.tile([128, CH], F32R, name=f"{name}b{n}", tag="qb")
                        nc.scalar.activation(qb[:, :], pp[:, :], AF.Identity,
                                             bias=(bq if name == "q" else bk))
                        sw = pools["tp"].tile([128, CH], F32, name=f"{name}sw{n}", tag="tp")
                        for j in range(CH // 512):
                            nc.tensor.matmul(sw[:, 512 * j:512 * (j + 1)], psw,
                                             qb[:, 512 * j:512 * (j + 1)],
                                             start=True, stop=True)
                        t1 = chunks.tile([128, CH], F32, name=f"{name}t1{n}", tag="t1")
                        t2 = chunks.tile([128, CH], F32, name=f"{name}t2{n}", tag="t2")
                        nc.vector.tensor_mul(t1[:, :], qb[:, :], cos_c[:, :])
                        nc.vector.tensor_mul(t2[:, :], sw[:, :], sin_c[:, :])
                        nc.vector.tensor_add(dst[:, tcol:tcol + CH], t1[:, :], t2[:, :])

            # ── phase 3: attention for one (batch, q-chunk), both heads
            # packed: the two heads' K=64 score matmuls run concurrently in
            # disjoint PE row-groups via tile_position, so all 128 array rows
            # stay active (HAM) and scores cost one matmul-time per pair.
            def emit_att2(b, qc):
                toff = b * S
                qcols = slice(toff + QW * qc, toff + QW * (qc + 1))
                ots = [pools["ot"].tile([HD + 1, QW], F32, name=f"ot{b}{h}{qc}", tag="ot")
                       for h in range(HPC)]

                def emit_scores(kt):
                    krows = slice(toff + 128 * kt, toff + 128 * (kt + 1))
                    exs = []
                    scs = []
                    for h in range(HPC):
                        po = HD * h
                        sc = pools["sc"].tile([128, QW], F32, name=f"sc{b}{h}{qc}{kt}", tag="sc")
                        nc.tensor.matmul(sc[:, :], ktr[po:po + HD, krows],
                                         qtr[po:po + HD, qcols], start=True, stop=True,
                                         tile_position=(po, 0))
                        scs.append(sc)
                    for h in range(HPC):
                        sc = scs[h]
                        if use_mask:
                            mtile = expp.tile([128, QW], F32, name=f"mt{b}{h}{qc}{kt}", tag="mt")
                            nc.sync.dma_start(
                                out=mtile,
                                in_=mt_d[128 * kt:128 * (kt + 1), QW * qc:QW * (qc + 1)])
                            nc.vector.tensor_scalar_mul(sc[:, :], sc[:, :], SCALE)
                            nc.vector.tensor_add(sc[:, :], sc[:, :], mtile[:, :])
                        ex = expp.tile([128, QW], BF16, name=f"ex{b}{h}{qc}{kt}", tag="ex")
                        nc.scalar.activation(ex[:, :], sc[:, :], AF.Exp,
                                             scale=(1.0 if use_mask else SCALE))
                        exs.append(ex)
                    return exs

                def emit_av(kt, exs):
                    kt_glob = (toff + 128 * kt) // 128
                    for h in range(HPC):
                        nc.tensor.matmul(ots[h][:, :],
                                         vnat[kt_glob][:, 65 * h:65 * h + HD + 1],
                                         exs[h][:, :], start=(kt == 0), stop=(kt == KT - 1))

                # software-pipelined: scores(kt+1) issue before AV(kt)
                exs = emit_scores(0)
                for kt in range(1, KT):
                    ex_next = emit_scores(kt)
                    emit_av(kt - 1, exs)
                    exs = ex_next
                emit_av(KT - 1, exs)
                # normalize: rows 0..63 divided by row 64
                for h in range(HPC):
                    po = HD * h
                    rec = outp.tile([1, QW], F32, name=f"rec{b}{h}{qc}", tag="rec")
                    nc.vector.reciprocal(rec[:, :], ots[h][HD:HD + 1, :])
                    bcast = outp.tile([HD, QW], F32, name=f"bc{b}{h}{qc}", tag="bc")
                    nc.gpsimd.partition_broadcast(bcast[:, :], rec[:, :])
                    nc.vector.tensor_mul(aoT[po:po + HD, qcols], ots[h][0:HD, :], bcast[:, :])

            # ── phase 4: partial output projection for one (batch,qchunk)
            def emit_oproj(b, qc):
                toff = b * S
                qcols = slice(toff + QW * qc, toff + QW * (qc + 1))
                for mo in range(D // 128):
                    yp = pools["proj"].tile([128, QW], F32, name=f"yp{b}{qc}{mo}", tag="proj")
                    nc.tensor.matmul(yp[:, :], wo[:, mo, :], aoT[:, qcols],
                                     start=True, stop=True)
                    yo = outp.tile([128, QW], F32, name=f"yo{b}{qc}{mo}", tag="yo")
                    nc.any.tensor_copy(yo[:, :], yp[:, :])
                    nc.sync.dma_start(out=ypT_d[128 * mo:128 * (mo + 1), qcols], in_=yo)

            # ── emission: sequential phases, phase-scoped psum pools ────
            ps1 = ExitStack()
            ps_tp = ps1.enter_context(tc.tile_pool(name="ps_tp", bufs=4, space="PSUM"))
            ps_proj = ps1.enter_context(tc.tile_pool(name="ps_proj", bufs=3, space="PSUM"))
            pools = {"tp": ps_tp, "proj": ps_proj}
            for n in range(NCHUNK):
                emit_chunk(n)
            ps1.close()
            ps3 = ExitStack()
            pools["sc"] = ps3.enter_context(tc.tile_pool(name="ps_sc", bufs=4, space="PSUM"))
            pools["ot"] = ps3.enter_context(tc.tile_pool(name="ps_ot", bufs=2, space="PSUM"))
            pools["proj"] = ps3.enter_context(tc.tile_pool(name="ps_y", bufs=2, space="PSUM"))
            for b in range(B):
                for qc in range(QC):
                    emit_att2(b, qc)
                    emit_oproj(b, qc)
            ps3.close()

    nc.compile()
    nc.m = get_hw_module(nc.m)
    return nc


def _get_nc(use_mask: bool):
    key = ("nc", use_mask)
    if key not in _CACHE:
        _CACHE[key] = _build(use_mask)
    return _CACHE[key]


def kernel(x, rope, mask, Wq, bq, Wk, bk, Wv, bv, Wo, bo, _trace=False):
    x = np.ascontiguousarray(np.asarray(x, dtype=np.float32))
    rope = np.asarray(rope, dtype=np.float32)
    mask = np.asarray(mask, dtype=np.float32)
    use_mask = bool(np.any(mask))

    x2d = x.reshape(T, D)
    cos = rope[0, 0, :, 0, :]                      # [S, 64]
    sin = rope[1, 0, :, 0, :]
    sgn = np.where(np.arange(HD) % 2 == 0, -1.0, 1.0).astype(np.float32)[:, None]
    cosT = np.ascontiguousarray(np.tile(cos.T, (HPC, B)))          # [128, T]
    sinT = np.ascontiguousarray(np.tile(sin.T * sgn, (HPC, B)))    # [128, T]
    psw = np.zeros((128, 128), dtype=np.float32)
    idx = np.arange(128)
    psw[idx ^ 1, idx] = 1.0

    nc = _get_nc(use_mask)
    in_maps = []
    for c in range(NC):
        cs = slice(CW * c, CW * (c + 1))
        m = dict(
            x=x2d,
            wq=np.ascontiguousarray(Wq[:, cs]), bq=np.ascontiguousarray(bq[cs]).reshape(CW, 1),
            wk=np.ascontiguousarray(Wk[:, cs]), bk=np.ascontiguousarray(bk[cs]).reshape(CW, 1),
            wv=np.ascontiguousarray(Wv[:, cs]), bv=np.ascontiguousarray(bv[cs]).reshape(CW, 1),
            wo=np.ascontiguousarray(Wo[cs, :]),
            cost=cosT, sint=sinT, pswap=psw,
        )
        if use_mask:
            m["maskt"] = np.ascontiguousarray(mask[0, 0].T)
        in_maps.append({k: np.asarray(v, dtype=np.float32) for k, v in m.items()})

    res = bass_utils.run_bass_kernel_spmd(
        nc, in_maps, core_ids=list(range(NC)), trace=_trace)
    # row-parallel unshard: sum the per-core partial projections, add bias
    ypT = res.results[0]["ypT"].astype(np.float32)
    for c in range(1, NC):
        ypT = ypT + res.results[c]["ypT"]
    out = (ypT.T + np.asarray(bo, dtype=np.float32)).reshape(B, S, D).astype(np.float32)
    out = np.ascontiguousarray(out)
    if _trace:
        return out, res
    return out


# revision 19
# speedup vs baseline: 1.2120x; 1.1058x over previous
"""Trainium2 Bass kernel for CosyVoice3 DiT attention (B=2, S=2048, H=16, hd=64, D=1024).

Sharding: tensor parallelism over heads — 2 heads per core on 8 cores.
Each core computes QKV projections for its head slice, RoPE, full attention
for its 2 heads, then its heads' contribution to the output projection
(row-parallel). The host gather sums the 8 partial outputs (the standard
row-parallel TP reduction) and adds the output bias.

Layout trick: everything is computed transposed ([dim, tokens]) so the
attention matmuls need no on-chip transposes of the big S x S matrices:
  scoresT[k,q] = K @ Q^T    (lhsT = K^T slice, rhs = Q^T slice)
  outT[d,q]    = V_aug^T @ expT  with V_aug = [V | ones] giving the softmax
                 denominator for free in row 64.
Softmax skips max-subtraction (scores are O(10) for this model family, and
exp is computed in fp32 which is safe up to ~88).

The emission order interleaves batch-0 attention with the tail of the
QKV-projection phase so the PE never idles long enough for the HAM clock
gate to re-throttle it to 1.2 GHz.
"""
import sys
sys.path.insert(0, "/opt/trn_rl_repo")
from contextlib import ExitStack
import numpy as np

# NTFF profile hook shim: this image's antenv lacks axon_hooks, which
# bass_utils imports unconditionally when trace=True (and the boot-time
# installer degrades silently without it). Recreate the module and install
# the ctypes-based hook so neuron-profile traces work.
import types as _types
try:
    import antenv as _antenv
    if "antenv.axon_hooks" not in sys.modules:
        _hooks = _types.ModuleType("antenv.axon_hooks")
        _hook_box = [None]
        _hooks.set_axon_ntff_profile_hook = lambda h: _hook_box.__setitem__(0, h)
        _hooks.get_axon_ntff_profile_hook = lambda: _hook_box[0]
        sys.modules["antenv.axon_hooks"] = _hooks
        _antenv.axon_hooks = _hooks
        try:
            from trn_agent_boot.trn_boot import _ntff_profile_via_ctypes
            _hooks.set_axon_ntff_profile_hook(
                _ntff_profile_via_ctypes("/opt/axon/libaxon_pjrt.so"))
        except Exception:
            pass
except Exception:
    pass

import concourse.bass as bass
import concourse.mybir as mybir
from concourse import bacc
from concourse.tile import TileContext
from concourse.bass_interp import get_hw_module
from concourse import bass_utils
from concourse.masks import make_identity
bass_utils.upload_artifacts = lambda tmpdir: str(tmpdir)  # no S3 in container

# ── constants (hardcoded per problem spec) ────────────────────────────────
B, S, D, H, HD = 2, 2048, 1024, 16, 64
T = B * S                 # 4096 tokens
NC = 8                    # cores
HPC = H // NC             # 2 heads per core
CW = HPC * HD             # 128 rows/cols per core
SCALE = 1.0 / np.sqrt(HD)
F32 = mybir.dt.float32
F32R = mybir.dt.float32r
BF16 = mybir.dt.bfloat16
AF = mybir.ActivationFunctionType

_CACHE = {}


def _build(use_mask: bool):
    nc = bacc.Bacc("TRN2", target_bir_lowering=False, debug=False, num_devices=NC)

    # inputs (per-core slices supplied by host)
    x_d = nc.dram_tensor("x", [T, D], F32, kind="ExternalInput")
    wq_d = nc.dram_tensor("wq", [D, CW], F32R, kind="ExternalInput")
    wk_d = nc.dram_tensor("wk", [D, CW], F32R, kind="ExternalInput")
    wv_d = nc.dram_tensor("wv", [D, CW], F32R, kind="ExternalInput")
    # wo: the CW rows of Wo owned by this core's heads -> [CW, D]
    wo_d = nc.dram_tensor("wo", [CW, D], F32R, kind="ExternalInput")
    bq_d = nc.dram_tensor("bq", [CW, 1], F32, kind="ExternalInput")
    bk_d = nc.dram_tensor("bk", [CW, 1], F32, kind="ExternalInput")
    bv_d = nc.dram_tensor("bv", [CW, 1], F32, kind="ExternalInput")
    cos_d = nc.dram_tensor("cost", [CW, T], F32, kind="ExternalInput")
    sin_d = nc.dram_tensor("sint", [CW, T], F32, kind="ExternalInput")   # sign-folded
    psw_d = nc.dram_tensor("pswap", [128, 128], F32R, kind="ExternalInput")
    if use_mask:
        mt_d = nc.dram_tensor("maskt", [S, S], F32, kind="ExternalInput")

    # partial output, transposed: ypT[n, t] = sum over this core's head dims
    ypT_d = nc.dram_tensor("ypT", [D, T], F32, kind="ExternalOutput")

    NCHUNK = 8            # token chunks of 512 for projections
    CH = T // NCHUNK      # 512
    KT = S // 128         # 16 k-tiles per batch
    QW = 512              # q chunk width
    QC = S // QW          # 4 q chunks per batch

    with TileContext(nc) as tc:
        with tc.tile_pool(name="persist", bufs=1) as persist, \
             tc.tile_pool(name="wpool", bufs=1) as wpool, \
             tc.tile_pool(name="xload", bufs=6) as xload, \
             tc.tile_pool(name="xtp", bufs=10) as xtpool, \
             tc.tile_pool(name="chunks", bufs=2) as chunks, \
             tc.tile_pool(name="expp", bufs=4) as expp, \
             tc.tile_pool(name="outp", bufs=3) as outp:

            # ── persistent tiles ────────────────────────────────────────
            ident = persist.tile([128, 128], F32, name="ident")
            make_identity(nc, ident)
            psw = persist.tile([128, 128], F32R, name="psw")
            nc.sync.dma_start(out=psw, in_=psw_d[:, :])
            wq = wpool.tile([128, D // 128, CW], F32R, name="wq_sb")
            wk = wpool.tile([128, D // 128, CW], F32R, name="wk_sb")
            wv = wpool.tile([128, D // 128, CW], F32R, name="wv_sb")
            for wt, wdr in ((wq, wq_d), (wk, wk_d), (wv, wv_d)):
                nc.sync.dma_start(out=wt, in_=wdr.ap().rearrange("(kc p) m -> p kc m", p=128))
            # wo rows for this core: [CW, D] -> lhsT chunks [CW, 128] per out-col group
            wo = wpool.tile([CW, D // 128, 128], F32R, name="wo_sb")
            nc.sync.dma_start(out=wo, in_=wo_d.ap().rearrange("p (mc m) -> p mc m", m=128))
            bq = wpool.tile([CW, 1], F32, name="bq_sb")
            bk = wpool.tile([CW, 1], F32, name="bk_sb")
            bv0 = wpool.tile([HD, 1], F32, name="bv0_sb")
            bv1 = wpool.tile([HD, 1], F32, name="bv1_sb")
            nc.sync.dma_start(out=bq, in_=bq_d[:, :])
            nc.sync.dma_start(out=bk, in_=bk_d[:, :])
            nc.sync.dma_start(out=bv0, in_=bv_d[0:HD, :])
            nc.sync.dma_start(out=bv1, in_=bv_d[HD:CW, :])

            qtr = persist.tile([128, T], BF16, name="qtr")    # rope'd Q^T
            ktr = persist.tile([128, T], BF16, name="ktr")    # rope'd K^T
            aoT = persist.tile([128, T], F32R, name="aoT")    # normalized attn out^T
            # V natural per k-tile: [128 tok, 2*(64+1)] with ones cols
            vnat = [persist.tile([128, 2 * (HD + 1)], BF16, name=f"vnat{i}")
                    for i in range(T // 128)]

            # ── phase 1: per token-chunk: transpose x, QKV proj, rope ───
            def emit_chunk(n):
                tcol = n * CH
                xts = [xtpool.tile([128, CH], F32R, name=f"xt{n}_{dc}", tag="xt")
                       for dc in range(D // 128)]
                xns = []
                for tt in range(CH // 128):
                    xn = xload.tile([128, D], F32, name=f"xn{n}_{tt}", tag="xn")
                    nc.sync.dma_start(out=xn, in_=x_d[tcol + 128 * tt: tcol + 128 * (tt + 1), :])
                    xns.append(xn)
                for dc in range(D // 128):
                    # pack the 4 token-block transposes of one d-block into one
                    # psum bank, one copy out
                    xp = pools["tp"].tile([128, CH], F32, name=f"xp{n}{dc}", tag="tp")
                    for tt in range(CH // 128):
                        nc.tensor.transpose(xp[:, 128 * tt:128 * (tt + 1)],
                                            xns[tt][:, 128 * dc:128 * (dc + 1)], ident)
                    nc.any.tensor_copy(xts[dc][:, :], xp[:, :])

                cos_c = chunks.tile([128, CH], F32, name=f"cos{n}", tag="cosc")
                sin_c = chunks.tile([128, CH], F32, name=f"sin{n}", tag="sinc")
                nc.sync.dma_start(out=cos_c, in_=cos_d[:, tcol:tcol + CH])
                nc.sync.dma_start(out=sin_c, in_=sin_d[:, tcol:tcol + CH])

                for name, wt, dst in (("q", wq, qtr), ("k", wk, ktr), ("v", wv, None)):
                    pp = pools["proj"].tile([128, CH], F32, name=f"{name}ps{n}", tag="proj")
                    for dc in range(D // 128):
                        nc.tensor.matmul(pp[:, :], wt[:, dc, :], xts[dc][:, :],
                                         start=(dc == 0), stop=(dc == D // 128 - 1))
                    if name == "v":
                        # per-head natural V via PE transpose; ones row becomes
                        # the denominator column after transpose
                        for h in range(HPC):
                            vth = chunks.tile([HD + 1, CH], F32, name=f"vt{n}{h}", tag="vth")
                            nc.scalar.activation(vth[0:HD, :], pp[HD * h:HD * (h + 1), :],
                                                 AF.Identity, bias=(bv0 if h == 0 else bv1))
                            nc.vector.memset(vth[HD:HD + 1, :], 1.0)
                            for ktl in range(CH // 128):
                                vp = pools["tp"].tile([128, HD + 1], F32, name=f"vp{n}{h}{ktl}", tag="tp")
                                nc.tensor.transpose(vp[:, :], vth[:, 128 * ktl:128 * (ktl + 1)],
                                                    ident[0:HD + 1, 0:HD + 1])
                                kt_glob = (tcol + 128 * ktl) // 128
                                nc.any.tensor_copy(
                                    vnat[kt_glob][:, 65 * h:65 * h + HD + 1], vp[:, :])
                    else:
                        # bias + rope: dst_chunk = (p+b)*cos + swap(p+b)*sin_signed
                        qb = chunks.tile([128, CH], F32R, name=f"{name}b{n}", tag="qb")
                        nc.scalar.activation(qb[:, :], pp[:, :], AF.Identity,
                                             bias=(bq if name == "q" else bk))
                        sw = pools["tp"].tile([128, CH], F32, name=f"{name}sw{n}", tag="tp")
                        for j in range(CH // 512):
                            nc.tensor.matmul(sw[:, 512 * j:512 * (j + 1)], psw,
                                             qb[:, 512 * j:512 * (j + 1)],
                                             start=True, stop=True)
                        t1 = chunks.tile([128, CH], F32, name=f"{name}t1{n}", tag="t1")
                        t2 = chunks.tile([128, CH], F32, name=f"{name}t2{n}", tag="t2")
                        nc.vector.tensor_mul(t1[:, :], qb[:, :], cos_c[:, :])
                        nc.vector.tensor_mul(t2[:, :], sw[:, :], sin_c[:, :])
                        nc.vector.tensor_add(dst[:, tcol:tcol + CH], t1[:, :], t2[:, :])

            # ── phase 3: attention for one (batch, q-chunk), both heads
            # packed: the two heads' K=64 score matmuls run concurrently in
            # disjoint PE row-groups via tile_position, so all 128 array rows
            # stay active (HAM) and scores cost one matmul-time per pair.
            def emit_att2(b, qc):
                toff = b * S
                qcols = slice(toff + QW * qc, toff + QW * (qc + 1))
                ots = [pools["ot"].tile([HD + 1, QW], F32, name=f"ot{b}{h}{qc}", tag="ot")
                       for h in range(HPC)]

                def emit_scores(kt):
                    krows = slice(toff + 128 * kt, toff + 128 * (kt + 1))
                    exs = []
                    scs = []
                    for h in range(HPC):
                        po = HD * h
                        sc = pools["sc"].tile([128, QW], F32, name=f"sc{b}{h}{qc}{kt}", tag="sc")
                        nc.tensor.matmul(sc[:, :], ktr[po:po + HD, krows],
                                         qtr[po:po + HD, qcols], start=True, stop=True,
                                         tile_position=(po, 0))
                        scs.append(sc)
                    for h in range(HPC):
                        sc = scs[h]
                        if use_mask:
                            mtile = expp.tile([128, QW], F32, name=f"mt{b}{h}{qc}{kt}", tag="mt")
                            nc.sync.dma_start(
                                out=mtile,
                                in_=mt_d[128 * kt:128 * (kt + 1), QW * qc:QW * (qc + 1)])
                            nc.vector.tensor_scalar_mul(sc[:, :], sc[:, :], SCALE)
                            nc.vector.tensor_add(sc[:, :], sc[:, :], mtile[:, :])
                        ex = expp.tile([128, QW], BF16, name=f"ex{b}{h}{qc}{kt}", tag="ex")
                        nc.scalar.activation(ex[:, :], sc[:, :], AF.Exp,
                                             scale=(1.0 if use_mask else SCALE))
                        exs.append(ex)
                    return exs

                def emit_av(kt, exs):
                    kt_glob = (toff + 128 * kt) // 128
                    for h in range(HPC):
                        nc.tensor.matmul(ots[h][:, :],
                                         vnat[kt_glob][:, 65 * h:65 * h + HD + 1],
                                         exs[h][:, :], start=(kt == 0), stop=(kt == KT - 1))

                # software-pipelined: scores(kt+1) issue before AV(kt)
                exs = emit_scores(0)
                for kt in range(1, KT):
                    ex_next = emit_scores(kt)
                    emit_av(kt - 1, exs)
                    exs = ex_next
                emit_av(KT - 1, exs)
                # normalize: rows 0..63 divided by row 64
                for h in range(HPC):
                    po = HD * h
                    rec = outp.tile([1, QW], F32, name=f"rec{b}{h}{qc}", tag="rec")
                    nc.vector.reciprocal(rec[:, :], ots[h][HD:HD + 1, :])
                    bcast = outp.tile([HD, QW], F32, name=f"bc{b}{h}{qc}", tag="bc")
                    nc.gpsimd.partition_broadcast(bcast[:, :], rec[:, :])
                    nc.vector.tensor_mul(aoT[po:po + HD, qcols], ots[h][0:HD, :], bcast[:, :])

            # ── phase 4: partial output projection for one (batch,qchunk)
            def emit_oproj(b, qc):
                toff = b * S
                qcols = slice(toff + QW * qc, toff + QW * (qc + 1))
                for mo in range(D // 128):
                    yp = pools["proj"].tile([128, QW], F32, name=f"yp{b}{qc}{mo}", tag="proj")
                    nc.tensor.matmul(yp[:, :], wo[:, mo, :], aoT[:, qcols],
                                     start=True, stop=True)
                    yo = outp.tile([128, QW], F32, name=f"yo{b}{qc}{mo}", tag="yo")
                    nc.any.tensor_copy(yo[:, :], yp[:, :])
                    nc.sync.dma_start(out=ypT_d[128 * mo:128 * (mo + 1), qcols], in_=yo)

            # ── emission: sequential phases, phase-scoped psum pools ────
            ps1 = ExitStack()
            ps_tp = ps1.enter_context(tc.tile_pool(name="ps_tp", bufs=4, space="PSUM"))
            ps_proj = ps1.enter_context(tc.tile_pool(name="ps_proj", bufs=3, space="PSUM"))
            pools = {"tp": ps_tp, "proj": ps_proj}
            for n in range(NCHUNK):
                emit_chunk(n)
            ps1.close()
            ps3 = ExitStack()
            pools["sc"] = ps3.enter_context(tc.tile_pool(name="ps_sc", bufs=3, space="PSUM"))
            pools["ot"] = ps3.enter_context(tc.tile_pool(name="ps_ot", bufs=3, space="PSUM"))
            pools["proj"] = ps3.enter_context(tc.tile_pool(name="ps_y", bufs=2, space="PSUM"))
            # O-proj lags one q-chunk behind attention so the PE never waits
            # on the normalize chain at group boundaries
            groups = [(b, qc) for b in range(B) for qc in range(QC)]
            prev = None
            for g in groups:
                emit_att2(*g)
                if prev is not None:
                    emit_oproj(*prev)
                prev = g
            emit_oproj(*prev)
            ps3.close()

    nc.compile()
    nc.m = get_hw_module(nc.m)
    return nc


def _get_nc(use_mask: bool):
    key = ("nc", use_mask)
    if key not in _CACHE:
        _CACHE[key] = _build(use_mask)
    return _CACHE[key]


def kernel(x, rope, mask, Wq, bq, Wk, bk, Wv, bv, Wo, bo, _trace=False):
    x = np.ascontiguousarray(np.asarray(x, dtype=np.float32))
    rope = np.asarray(rope, dtype=np.float32)
    mask = np.asarray(mask, dtype=np.float32)
    use_mask = bool(np.any(mask))

    x2d = x.reshape(T, D)
    cos = rope[0, 0, :, 0, :]                      # [S, 64]
    sin = rope[1, 0, :, 0, :]
    sgn = np.where(np.arange(HD) % 2 == 0, -1.0, 1.0).astype(np.float32)[:, None]
    cosT = np.ascontiguousarray(np.tile(cos.T, (HPC, B)))          # [128, T]
    sinT = np.ascontiguousarray(np.tile(sin.T * sgn, (HPC, B)))    # [128, T]
    psw = np.zeros((128, 128), dtype=np.float32)
    idx = np.arange(128)
    psw[idx ^ 1, idx] = 1.0

    nc = _get_nc(use_mask)
    in_maps = []
    for c in range(NC):
        cs = slice(CW * c, CW * (c + 1))
        m = dict(
            x=x2d,
            wq=np.ascontiguousarray(Wq[:, cs]), bq=np.ascontiguousarray(bq[cs]).reshape(CW, 1),
            wk=np.ascontiguousarray(Wk[:, cs]), bk=np.ascontiguousarray(bk[cs]).reshape(CW, 1),
            wv=np.ascontiguousarray(Wv[:, cs]), bv=np.ascontiguousarray(bv[cs]).reshape(CW, 1),
            wo=np.ascontiguousarray(Wo[cs, :]),
            cost=cosT, sint=sinT, pswap=psw,
        )
        if use_mask:
            m["maskt"] = np.ascontiguousarray(mask[0, 0].T)
        in_maps.append({k: np.asarray(v, dtype=np.float32) for k, v in m.items()})

    res = bass_utils.run_bass_kernel_spmd(
        nc, in_maps, core_ids=list(range(NC)), trace=_trace)
    # row-parallel unshard: sum the per-core partial projections, add bias
    ypT = res.results[0]["ypT"].astype(np.float32)
    for c in range(1, NC):
        ypT = ypT + res.results[c]["ypT"]
    out = (ypT.T + np.asarray(bo, dtype=np.float32)).reshape(B, S, D).astype(np.float32)
    out = np.ascontiguousarray(out)
    if _trace:
        return out, res
    return out


# revision 20
# speedup vs baseline: 1.2584x; 1.0382x over previous
"""Trainium2 Bass kernel for CosyVoice3 DiT attention (B=2, S=2048, H=16, hd=64, D=1024).

Sharding: tensor parallelism over heads — 2 heads per core on 8 cores.
Each core computes QKV projections for its head slice, RoPE, full attention
for its 2 heads, then its heads' contribution to the output projection
(row-parallel). The host gather sums the 8 partial outputs (the standard
row-parallel TP reduction) and adds the output bias.

Layout trick: everything is computed transposed ([dim, tokens]) so the
attention matmuls need no on-chip transposes of the big S x S matrices:
  scoresT[k,q] = K @ Q^T    (lhsT = K^T slice, rhs = Q^T slice)
  outT[d,q]    = V_aug^T @ expT  with V_aug = [V | ones] giving the softmax
                 denominator for free in row 64.
Softmax skips max-subtraction (scores are O(10) for this model family, and
exp is computed in fp32 which is safe up to ~88).

The emission order interleaves batch-0 attention with the tail of the
QKV-projection phase so the PE never idles long enough for the HAM clock
gate to re-throttle it to 1.2 GHz.
"""
import sys
sys.path.insert(0, "/opt/trn_rl_repo")
from contextlib import ExitStack
import numpy as np

# NTFF profile hook shim: this image's antenv lacks axon_hooks, which
# bass_utils imports unconditionally when trace=True (and the boot-time
# installer degrades silently without it). Recreate the module and install
# the ctypes-based hook so neuron-profile traces work.
import types as _types
try:
    import antenv as _antenv
    if "antenv.axon_hooks" not in sys.modules:
        _hooks = _types.ModuleType("antenv.axon_hooks")
        _hook_box = [None]
        _hooks.set_axon_ntff_profile_hook = lambda h: _hook_box.__setitem__(0, h)
        _hooks.get_axon_ntff_profile_hook = lambda: _hook_box[0]
        sys.modules["antenv.axon_hooks"] = _hooks
        _antenv.axon_hooks = _hooks
        try:
            from trn_agent_boot.trn_boot import _ntff_profile_via_ctypes
            _hooks.set_axon_ntff_profile_hook(
                _ntff_profile_via_ctypes("/opt/axon/libaxon_pjrt.so"))
        except Exception:
            pass
except Exception:
    pass

import concourse.bass as bass
import concourse.mybir as mybir
from concourse import bacc
from concourse.tile import TileContext
from concourse.bass_interp import get_hw_module
from concourse import bass_utils
from concourse.masks import make_identity
bass_utils.upload_artifacts = lambda tmpdir: str(tmpdir)  # no S3 in container

# ── constants (hardcoded per problem spec) ────────────────────────────────
B, S, D, H, HD = 2, 2048, 1024, 16, 64
T = B * S                 # 4096 tokens
NC = 8                    # cores
HPC = H // NC             # 2 heads per core
CW = HPC * HD             # 128 rows/cols per core
SCALE = 1.0 / np.sqrt(HD)
F32 = mybir.dt.float32
F32R = mybir.dt.float32r
BF16 = mybir.dt.bfloat16
AF = mybir.ActivationFunctionType

_CACHE = {}


def _build(use_mask: bool):
    nc = bacc.Bacc("TRN2", target_bir_lowering=False, debug=False, num_devices=NC)

    # inputs (per-core slices supplied by host)
    x_d = nc.dram_tensor("x", [T, D], F32, kind="ExternalInput")
    wq_d = nc.dram_tensor("wq", [D, CW], F32R, kind="ExternalInput")
    wk_d = nc.dram_tensor("wk", [D, CW], F32R, kind="ExternalInput")
    wv_d = nc.dram_tensor("wv", [D, CW], F32R, kind="ExternalInput")
    # wo: the CW rows of Wo owned by this core's heads -> [CW, D]
    wo_d = nc.dram_tensor("wo", [CW, D], F32R, kind="ExternalInput")
    bq_d = nc.dram_tensor("bq", [CW, 1], F32, kind="ExternalInput")
    bk_d = nc.dram_tensor("bk", [CW, 1], F32, kind="ExternalInput")
    bv_d = nc.dram_tensor("bv", [CW, 1], F32, kind="ExternalInput")
    cos_d = nc.dram_tensor("cost", [CW, T], F32, kind="ExternalInput")
    sin_d = nc.dram_tensor("sint", [CW, T], F32, kind="ExternalInput")   # sign-folded
    psw_d = nc.dram_tensor("pswap", [128, 128], F32R, kind="ExternalInput")
    if use_mask:
        mt_d = nc.dram_tensor("maskt", [S, S], F32, kind="ExternalInput")

    # partial output, transposed: ypT[n, t] = sum over this core's head dims
    ypT_d = nc.dram_tensor("ypT", [D, T], F32, kind="ExternalOutput")

    NCHUNK = 8            # token chunks of 512 for projections
    CH = T // NCHUNK      # 512
    KT = S // 128         # 16 k-tiles per batch
    QW = 512              # q chunk width
    QC = S // QW          # 4 q chunks per batch

    with TileContext(nc) as tc:
        with tc.tile_pool(name="persist", bufs=1) as persist, \
             tc.tile_pool(name="wpool", bufs=1) as wpool, \
             tc.tile_pool(name="xload", bufs=6) as xload, \
             tc.tile_pool(name="xtp", bufs=10) as xtpool, \
             tc.tile_pool(name="chunks", bufs=2) as chunks, \
             tc.tile_pool(name="expp", bufs=4) as expp, \
             tc.tile_pool(name="outp", bufs=3) as outp:

            # ── persistent tiles ────────────────────────────────────────
            ident = persist.tile([128, 128], F32, name="ident")
            make_identity(nc, ident)
            psw = persist.tile([128, 128], F32R, name="psw")
            nc.sync.dma_start(out=psw, in_=psw_d[:, :])
            wq = wpool.tile([128, D // 128, CW], F32R, name="wq_sb")
            wk = wpool.tile([128, D // 128, CW], F32R, name="wk_sb")
            wv = wpool.tile([128, D // 128, CW], F32R, name="wv_sb")
            for wt, wdr in ((wq, wq_d), (wk, wk_d), (wv, wv_d)):
                nc.sync.dma_start(out=wt, in_=wdr.ap().rearrange("(kc p) m -> p kc m", p=128))
            # wo rows for this core: [CW, D] -> lhsT chunks [CW, 128] per out-col group
            wo = wpool.tile([CW, D // 128, 128], F32R, name="wo_sb")
            nc.sync.dma_start(out=wo, in_=wo_d.ap().rearrange("p (mc m) -> p mc m", m=128))
            bq = wpool.tile([CW, 1], F32, name="bq_sb")
            bk = wpool.tile([CW, 1], F32, name="bk_sb")
            bv0 = wpool.tile([HD, 1], F32, name="bv0_sb")
            bv1 = wpool.tile([HD, 1], F32, name="bv1_sb")
            nc.sync.dma_start(out=bq, in_=bq_d[:, :])
            nc.sync.dma_start(out=bk, in_=bk_d[:, :])
            nc.sync.dma_start(out=bv0, in_=bv_d[0:HD, :])
            nc.sync.dma_start(out=bv1, in_=bv_d[HD:CW, :])

            qtr = persist.tile([128, T], BF16, name="qtr")    # rope'd Q^T
            ktr = persist.tile([128, T], BF16, name="ktr")    # rope'd K^T
            aoT = persist.tile([128, T], F32R, name="aoT")    # normalized attn out^T
            # V natural per k-tile: [128 tok, 2*(64+1)] with ones cols
            vnat = [persist.tile([128, 2 * (HD + 1)], BF16, name=f"vnat{i}")
                    for i in range(T // 128)]

            # ── phase 1: per token-chunk: transpose x, QKV proj, rope ───
            def emit_chunk(n):
                tcol = n * CH
                xts = [xtpool.tile([128, CH], F32R, name=f"xt{n}_{dc}", tag="xt")
                       for dc in range(D // 128)]
                xns = []
                for tt in range(CH // 128):
                    xn = xload.tile([128, D], F32, name=f"xn{n}_{tt}", tag="xn")
                    nc.sync.dma_start(out=xn, in_=x_d[tcol + 128 * tt: tcol + 128 * (tt + 1), :])
                    xns.append(xn)
                for dc in range(D // 128):
                    # pack the 4 token-block transposes of one d-block into one
                    # psum bank, one copy out
                    xp = pools["tp"].tile([128, CH], F32, name=f"xp{n}{dc}", tag="tp")
                    for tt in range(CH // 128):
                        nc.tensor.transpose(xp[:, 128 * tt:128 * (tt + 1)],
                                            xns[tt][:, 128 * dc:128 * (dc + 1)], ident)
                    nc.any.tensor_copy(xts[dc][:, :], xp[:, :])

                cos_c = chunks.tile([128, CH], F32, name=f"cos{n}", tag="cosc")
                sin_c = chunks.tile([128, CH], F32, name=f"sin{n}", tag="sinc")
                nc.sync.dma_start(out=cos_c, in_=cos_d[:, tcol:tcol + CH])
                nc.sync.dma_start(out=sin_c, in_=sin_d[:, tcol:tcol + CH])

                for name, wt, dst in (("q", wq, qtr), ("k", wk, ktr), ("v", wv, None)):
                    pp = pools["proj"].tile([128, CH], F32, name=f"{name}ps{n}", tag="proj")
                    for dc in range(D // 128):
                        nc.tensor.matmul(pp[:, :], wt[:, dc, :], xts[dc][:, :],
                                         start=(dc == 0), stop=(dc == D // 128 - 1))
                    if name == "v":
                        # per-head natural V via PE transpose; ones row becomes
                        # the denominator column after transpose
                        for h in range(HPC):
                            vth = chunks.tile([HD + 1, CH], F32, name=f"vt{n}{h}", tag="vth")
                            nc.scalar.activation(vth[0:HD, :], pp[HD * h:HD * (h + 1), :],
                                                 AF.Identity, bias=(bv0 if h == 0 else bv1))
                            nc.vector.memset(vth[HD:HD + 1, :], 1.0)
                            for ktl in range(CH // 128):
                                vp = pools["tp"].tile([128, HD + 1], F32, name=f"vp{n}{h}{ktl}", tag="tp")
                                nc.tensor.transpose(vp[:, :], vth[:, 128 * ktl:128 * (ktl + 1)],
                                                    ident[0:HD + 1, 0:HD + 1])
                                kt_glob = (tcol + 128 * ktl) // 128
                                nc.any.tensor_copy(
                                    vnat[kt_glob][:, 65 * h:65 * h + HD + 1], vp[:, :])
                    else:
                        # bias + rope: dst_chunk = (p+b)*cos + swap(p+b)*sin_signed
                        qb = chunks.tile([128, CH], F32R, name=f"{name}b{n}", tag="qb")
                        nc.scalar.activation(qb[:, :], pp[:, :], AF.Identity,
                                             bias=(bq if name == "q" else bk))
                        sw = pools["tp"].tile([128, CH], F32, name=f"{name}sw{n}", tag="tp")
                        for j in range(CH // 512):
                            nc.tensor.matmul(sw[:, 512 * j:512 * (j + 1)], psw,
                                             qb[:, 512 * j:512 * (j + 1)],
                                             start=True, stop=True)
                        t1 = chunks.tile([128, CH], F32, name=f"{name}t1{n}", tag="t1")
                        t2 = chunks.tile([128, CH], F32, name=f"{name}t2{n}", tag="t2")
                        nc.vector.tensor_mul(t1[:, :], qb[:, :], cos_c[:, :])
                        nc.vector.tensor_mul(t2[:, :], sw[:, :], sin_c[:, :])
                        nc.vector.tensor_add(dst[:, tcol:tcol + CH], t1[:, :], t2[:, :])

            # ── phase 3: attention for one (batch, q-chunk), both heads
            # packed: the two heads' K=64 score matmuls run concurrently in
            # disjoint PE row-groups via tile_position, so all 128 array rows
            # stay active (HAM) and scores cost one matmul-time per pair.
            def emit_att2(b, qc):
                toff = b * S
                qcols = slice(toff + QW * qc, toff + QW * (qc + 1))
                ots = [pools["ot"].tile([HD + 1, QW], F32, name=f"ot{b}{h}{qc}", tag="ot")
                       for h in range(HPC)]

                def emit_scores(kt):
                    krows = slice(toff + 128 * kt, toff + 128 * (kt + 1))
                    exs = []
                    scs = []
                    for h in range(HPC):
                        po = HD * h
                        sc = pools["sc"].tile([128, QW], F32, name=f"sc{b}{h}{qc}{kt}", tag="sc")
                        nc.tensor.matmul(sc[:, :], ktr[po:po + HD, krows],
                                         qtr[po:po + HD, qcols], start=True, stop=True,
                                         tile_position=(po, 0))
                        scs.append(sc)
                    for h in range(HPC):
                        sc = scs[h]
                        if use_mask:
                            mtile = expp.tile([128, QW], F32, name=f"mt{b}{h}{qc}{kt}", tag="mt")
                            nc.sync.dma_start(
                                out=mtile,
                                in_=mt_d[128 * kt:128 * (kt + 1), QW * qc:QW * (qc + 1)])
                            nc.vector.tensor_scalar_mul(sc[:, :], sc[:, :], SCALE)
                            nc.vector.tensor_add(sc[:, :], sc[:, :], mtile[:, :])
                        ex = expp.tile([128, QW], BF16, name=f"ex{b}{h}{qc}{kt}", tag="ex")
                        nc.scalar.activation(ex[:, :], sc[:, :], AF.Exp,
                                             scale=(1.0 if use_mask else SCALE))
                        exs.append(ex)
                    return exs

                def emit_av(kt, exs):
                    kt_glob = (toff + 128 * kt) // 128
                    for h in range(HPC):
                        nc.tensor.matmul(ots[h][:, :],
                                         vnat[kt_glob][:, 65 * h:65 * h + HD + 1],
                                         exs[h][:, :], start=(kt == 0), stop=(kt == KT - 1))

                # software-pipelined: scores(kt+1) issue before AV(kt)
                exs = emit_scores(0)
                for kt in range(1, KT):
                    ex_next = emit_scores(kt)
                    emit_av(kt - 1, exs)
                    exs = ex_next
                emit_av(KT - 1, exs)
                # normalize: rows 0..63 divided by row 64
                for h in range(HPC):
                    po = HD * h
                    den = outp.tile([1, QW], F32, name=f"den{b}{h}{qc}", tag="den")
                    nc.scalar.copy(den[:, :], ots[h][HD:HD + 1, :])
                    bcast = outp.tile([HD, QW], F32, name=f"bc{b}{h}{qc}", tag="bc")
                    nc.gpsimd.partition_broadcast(bcast[:, :], den[:, :])
                    rcp = outp.tile([HD, QW], F32, name=f"rcp{b}{h}{qc}", tag="rcp")
                    nc.vector.reciprocal_approx_fast(rcp[:, :], bcast[:, :])
                    nc.vector.tensor_mul(aoT[po:po + HD, qcols], ots[h][0:HD, :], rcp[:, :])

            # ── phase 4: partial output projection for one (batch,qchunk)
            def emit_oproj(b, qc):
                toff = b * S
                qcols = slice(toff + QW * qc, toff + QW * (qc + 1))
                for mo in range(D // 128):
                    yp = pools["proj"].tile([128, QW], F32, name=f"yp{b}{qc}{mo}", tag="proj")
                    nc.tensor.matmul(yp[:, :], wo[:, mo, :], aoT[:, qcols],
                                     start=True, stop=True)
                    yo = outp.tile([128, QW], F32, name=f"yo{b}{qc}{mo}", tag="yo")
                    nc.any.tensor_copy(yo[:, :], yp[:, :])
                    nc.sync.dma_start(out=ypT_d[128 * mo:128 * (mo + 1), qcols], in_=yo)

            # ── emission: sequential phases, phase-scoped psum pools ────
            ps1 = ExitStack()
            ps_tp = ps1.enter_context(tc.tile_pool(name="ps_tp", bufs=4, space="PSUM"))
            ps_proj = ps1.enter_context(tc.tile_pool(name="ps_proj", bufs=3, space="PSUM"))
            pools = {"tp": ps_tp, "proj": ps_proj}
            for n in range(NCHUNK):
                emit_chunk(n)
            ps1.close()
            ps3 = ExitStack()
            pools["sc"] = ps3.enter_context(tc.tile_pool(name="ps_sc", bufs=3, space="PSUM"))
            pools["ot"] = ps3.enter_context(tc.tile_pool(name="ps_ot", bufs=3, space="PSUM"))
            pools["proj"] = ps3.enter_context(tc.tile_pool(name="ps_y", bufs=2, space="PSUM"))
            # O-proj lags one q-chunk behind attention so the PE never waits
            # on the normalize chain at group boundaries
            groups = [(b, qc) for b in range(B) for qc in range(QC)]
            prev = None
            for g in groups:
                emit_att2(*g)
                if prev is not None:
                    emit_oproj(*prev)
                prev = g
            emit_oproj(*prev)
            ps3.close()

    nc.compile()
    nc.m = get_hw_module(nc.m)
    return nc


def _get_nc(use_mask: bool):
    key = ("nc", use_mask)
    if key not in _CACHE:
        _CACHE[key] = _build(use_mask)
    return _CACHE[key]


def kernel(x, rope, mask, Wq, bq, Wk, bk, Wv, bv, Wo, bo, _trace=False):
    x = np.ascontiguousarray(np.asarray(x, dtype=np.float32))
    rope = np.asarray(rope, dtype=np.float32)
    mask = np.asarray(mask, dtype=np.float32)
    use_mask = bool(np.any(mask))

    x2d = x.reshape(T, D)
    cos = rope[0, 0, :, 0, :]                      # [S, 64]
    sin = rope[1, 0, :, 0, :]
    sgn = np.where(np.arange(HD) % 2 == 0, -1.0, 1.0).astype(np.float32)[:, None]
    cosT = np.ascontiguousarray(np.tile(cos.T, (HPC, B)))          # [128, T]
    sinT = np.ascontiguousarray(np.tile(sin.T * sgn, (HPC, B)))    # [128, T]
    psw = np.zeros((128, 128), dtype=np.float32)
    idx = np.arange(128)
    psw[idx ^ 1, idx] = 1.0

    nc = _get_nc(use_mask)
    in_maps = []
    for c in range(NC):
        cs = slice(CW * c, CW * (c + 1))
        m = dict(
            x=x2d,
            wq=np.ascontiguousarray(Wq[:, cs]), bq=np.ascontiguousarray(bq[cs]).reshape(CW, 1),
            wk=np.ascontiguousarray(Wk[:, cs]), bk=np.ascontiguousarray(bk[cs]).reshape(CW, 1),
            wv=np.ascontiguousarray(Wv[:, cs]), bv=np.ascontiguousarray(bv[cs]).reshape(CW, 1),
            wo=np.ascontiguousarray(Wo[cs, :]),
            cost=cosT, sint=sinT, pswap=psw,
        )
        if use_mask:
            m["maskt"] = np.ascontiguousarray(mask[0, 0].T)
        in_maps.append({k: np.asarray(v, dtype=np.float32) for k, v in m.items()})

    res = bass_utils.run_bass_kernel_spmd(
        nc, in_maps, core_ids=list(range(NC)), trace=_trace)
    # row-parallel unshard: sum the per-core partial projections, add bias
    ypT = res.results[0]["ypT"].astype(np.float32)
    for c in range(1, NC):
        ypT = ypT + res.results[c]["ypT"]
    out = (ypT.T + np.asarray(bo, dtype=np.float32)).reshape(B, S, D).astype(np.float32)
    out = np.ascontiguousarray(out)
    if _trace:
        return out, res
    return out


# revision 21
# speedup vs baseline: 1.3170x; 1.0466x over previous
"""Trainium2 Bass kernel for CosyVoice3 DiT attention (B=2, S=2048, H=16, hd=64, D=1024).

Sharding: tensor parallelism over heads — 2 heads per core on 8 cores.
Each core computes QKV projections for its head slice, RoPE, full attention
for its 2 heads, then its heads' contribution to the output projection
(row-parallel). The host gather sums the 8 partial outputs (the standard
row-parallel TP reduction) and adds the output bias.

Layout trick: everything is computed transposed ([dim, tokens]) so the
attention matmuls need no on-chip transposes of the big S x S matrices:
  scoresT[k,q] = K @ Q^T    (lhsT = K^T slice, rhs = Q^T slice)
  outT[d,q]    = V_aug^T @ expT  with V_aug = [V | ones] giving the softmax
                 denominator for free in row 64.
Softmax skips max-subtraction (scores are O(10) for this model family, and
exp is computed in fp32 which is safe up to ~88).

The emission order interleaves batch-0 attention with the tail of the
QKV-projection phase so the PE never idles long enough for the HAM clock
gate to re-throttle it to 1.2 GHz.
"""
import sys
sys.path.insert(0, "/opt/trn_rl_repo")
from contextlib import ExitStack
import numpy as np

# NTFF profile hook shim: this image's antenv lacks axon_hooks, which
# bass_utils imports unconditionally when trace=True (and the boot-time
# installer degrades silently without it). Recreate the module and install
# the ctypes-based hook so neuron-profile traces work.
import types as _types
try:
    import antenv as _antenv
    if "antenv.axon_hooks" not in sys.modules:
        _hooks = _types.ModuleType("antenv.axon_hooks")
        _hook_box = [None]
        _hooks.set_axon_ntff_profile_hook = lambda h: _hook_box.__setitem__(0, h)
        _hooks.get_axon_ntff_profile_hook = lambda: _hook_box[0]
        sys.modules["antenv.axon_hooks"] = _hooks
        _antenv.axon_hooks = _hooks
        try:
            from trn_agent_boot.trn_boot import _ntff_profile_via_ctypes
            _hooks.set_axon_ntff_profile_hook(
                _ntff_profile_via_ctypes("/opt/axon/libaxon_pjrt.so"))
        except Exception:
            pass
except Exception:
    pass

import concourse.bass as bass
import concourse.mybir as mybir
from concourse import bacc
from concourse.tile import TileContext
from concourse.bass_interp import get_hw_module
from concourse import bass_utils
from concourse.masks import make_identity
bass_utils.upload_artifacts = lambda tmpdir: str(tmpdir)  # no S3 in container

# ── constants (hardcoded per problem spec) ────────────────────────────────
B, S, D, H, HD = 2, 2048, 1024, 16, 64
T = B * S                 # 4096 tokens
NC = 8                    # cores
HPC = H // NC             # 2 heads per core
CW = HPC * HD             # 128 rows/cols per core
SCALE = 1.0 / np.sqrt(HD)
F32 = mybir.dt.float32
F32R = mybir.dt.float32r
BF16 = mybir.dt.bfloat16
AF = mybir.ActivationFunctionType

_CACHE = {}


def _build(use_mask: bool):
    nc = bacc.Bacc("TRN2", target_bir_lowering=False, debug=False, num_devices=NC)

    # inputs (per-core slices supplied by host)
    x_d = nc.dram_tensor("x", [T, D], F32, kind="ExternalInput")
    wq_d = nc.dram_tensor("wq", [D, CW], F32R, kind="ExternalInput")
    wk_d = nc.dram_tensor("wk", [D, CW], F32R, kind="ExternalInput")
    wv_d = nc.dram_tensor("wv", [D, CW], F32R, kind="ExternalInput")
    # wo: the CW rows of Wo owned by this core's heads -> [CW, D]
    wo_d = nc.dram_tensor("wo", [CW, D], F32R, kind="ExternalInput")
    bq_d = nc.dram_tensor("bq", [CW, 1], F32, kind="ExternalInput")
    bk_d = nc.dram_tensor("bk", [CW, 1], F32, kind="ExternalInput")
    bv_d = nc.dram_tensor("bv", [CW, 1], F32, kind="ExternalInput")
    cos_d = nc.dram_tensor("cost", [CW, T], F32, kind="ExternalInput")
    sin_d = nc.dram_tensor("sint", [CW, T], F32, kind="ExternalInput")   # sign-folded
    psw_d = nc.dram_tensor("pswap", [128, 128], F32R, kind="ExternalInput")
    if use_mask:
        mt_d = nc.dram_tensor("maskt", [S, S], F32, kind="ExternalInput")

    # partial output, transposed: ypT[n, t] = sum over this core's head dims
    ypT_d = nc.dram_tensor("ypT", [D, T], F32, kind="ExternalOutput")

    NCHUNK = 8            # token chunks of 512 for projections
    CH = T // NCHUNK      # 512
    KT = S // 128         # 16 k-tiles per batch
    QW = 512              # q chunk width
    QC = S // QW          # 4 q chunks per batch

    with TileContext(nc) as tc:
        with tc.tile_pool(name="persist", bufs=1) as persist, \
             tc.tile_pool(name="wpool", bufs=1) as wpool, \
             tc.tile_pool(name="xload", bufs=6) as xload, \
             tc.tile_pool(name="xtp", bufs=10) as xtpool, \
             tc.tile_pool(name="chunks", bufs=2) as chunks, \
             tc.tile_pool(name="expp", bufs=4) as expp, \
             tc.tile_pool(name="outp", bufs=3) as outp:

            # ── persistent tiles ────────────────────────────────────────
            ident = persist.tile([128, 128], F32, name="ident")
            make_identity(nc, ident)
            psw = persist.tile([128, 128], F32R, name="psw")
            nc.sync.dma_start(out=psw, in_=psw_d[:, :])
            wq = wpool.tile([128, D // 128, CW], F32R, name="wq_sb")
            wk = wpool.tile([128, D // 128, CW], F32R, name="wk_sb")
            wv = wpool.tile([128, D // 128, CW], F32R, name="wv_sb")
            for wt, wdr in ((wq, wq_d), (wk, wk_d), (wv, wv_d)):
                nc.sync.dma_start(out=wt, in_=wdr.ap().rearrange("(kc p) m -> p kc m", p=128))
            # wo rows for this core: [CW, D] -> lhsT chunks [CW, 128] per out-col group
            wo = wpool.tile([CW, D // 128, 128], F32R, name="wo_sb")
            nc.sync.dma_start(out=wo, in_=wo_d.ap().rearrange("p (mc m) -> p mc m", m=128))
            bq = wpool.tile([CW, 1], F32, name="bq_sb")
            bk = wpool.tile([CW, 1], F32, name="bk_sb")
            bv0 = wpool.tile([HD, 1], F32, name="bv0_sb")
            bv1 = wpool.tile([HD, 1], F32, name="bv1_sb")
            nc.sync.dma_start(out=bq, in_=bq_d[:, :])
            nc.sync.dma_start(out=bk, in_=bk_d[:, :])
            nc.sync.dma_start(out=bv0, in_=bv_d[0:HD, :])
            nc.sync.dma_start(out=bv1, in_=bv_d[HD:CW, :])

            qtr = persist.tile([128, T], BF16, name="qtr")    # rope'd Q^T
            ktr = persist.tile([128, T], BF16, name="ktr")    # rope'd K^T
            aoT = persist.tile([128, T], F32R, name="aoT")    # normalized attn out^T
            # V natural per k-tile: [128 tok, 2*(64+1)] with ones cols
            vnat = [persist.tile([128, 2 * (HD + 1)], BF16, name=f"vnat{i}")
                    for i in range(T // 128)]

            # ── phase 1: per token-chunk: transpose x, QKV proj, rope ───
            def emit_chunk(n):
                tcol = n * CH
                xts = [xtpool.tile([128, CH], F32R, name=f"xt{n}_{dc}", tag="xt")
                       for dc in range(D // 128)]
                xns = []
                for tt in range(CH // 128):
                    xn = xload.tile([128, D], F32, name=f"xn{n}_{tt}", tag="xn")
                    nc.sync.dma_start(out=xn, in_=x_d[tcol + 128 * tt: tcol + 128 * (tt + 1), :])
                    xns.append(xn)
                for dc in range(D // 128):
                    # pack the 4 token-block transposes of one d-block into one
                    # psum bank, one copy out
                    xp = pools["tp"].tile([128, CH], F32, name=f"xp{n}{dc}", tag="tp")
                    for tt in range(CH // 128):
                        nc.tensor.transpose(xp[:, 128 * tt:128 * (tt + 1)],
                                            xns[tt][:, 128 * dc:128 * (dc + 1)], ident)
                    nc.any.tensor_copy(xts[dc][:, :], xp[:, :])

                cos_c = chunks.tile([128, CH], F32, name=f"cos{n}", tag="cosc")
                sin_c = chunks.tile([128, CH], F32, name=f"sin{n}", tag="sinc")
                nc.sync.dma_start(out=cos_c, in_=cos_d[:, tcol:tcol + CH])
                nc.sync.dma_start(out=sin_c, in_=sin_d[:, tcol:tcol + CH])

                for name, wt, dst in (("q", wq, qtr), ("k", wk, ktr), ("v", wv, None)):
                    pp = pools["proj"].tile([128, CH], F32, name=f"{name}ps{n}", tag="proj")
                    for dc in range(D // 128):
                        nc.tensor.matmul(pp[:, :], wt[:, dc, :], xts[dc][:, :],
                                         start=(dc == 0), stop=(dc == D // 128 - 1))
                    if name == "v":
                        # per-head natural V via PE transpose; ones row becomes
                        # the denominator column after transpose
                        for h in range(HPC):
                            vth = chunks.tile([HD + 1, CH], F32, name=f"vt{n}{h}", tag="vth")
                            nc.scalar.activation(vth[0:HD, :], pp[HD * h:HD * (h + 1), :],
                                                 AF.Identity, bias=(bv0 if h == 0 else bv1))
                            nc.vector.memset(vth[HD:HD + 1, :], 1.0)
                            for ktl in range(CH // 128):
                                vp = pools["tp"].tile([128, HD + 1], F32, name=f"vp{n}{h}{ktl}", tag="tp")
                                nc.tensor.transpose(vp[:, :], vth[:, 128 * ktl:128 * (ktl + 1)],
                                                    ident[0:HD + 1, 0:HD + 1])
                                kt_glob = (tcol + 128 * ktl) // 128
                                nc.any.tensor_copy(
                                    vnat[kt_glob][:, 65 * h:65 * h + HD + 1], vp[:, :])
                    else:
                        # bias + rope: dst_chunk = (p+b)*cos + swap(p+b)*sin_signed
                        qb = chunks.tile([128, CH], F32R, name=f"{name}b{n}", tag="qb")
                        nc.scalar.activation(qb[:, :], pp[:, :], AF.Identity,
                                             bias=(bq if name == "q" else bk))
                        sw = pools["tp"].tile([128, CH], F32, name=f"{name}sw{n}", tag="tp")
                        for j in range(CH // 512):
                            nc.tensor.matmul(sw[:, 512 * j:512 * (j + 1)], psw,
                                             qb[:, 512 * j:512 * (j + 1)],
                                             start=True, stop=True)
                        t1 = chunks.tile([128, CH], F32, name=f"{name}t1{n}", tag="t1")
                        t2 = chunks.tile([128, CH], F32, name=f"{name}t2{n}", tag="t2")
                        nc.vector.tensor_mul(t1[:, :], qb[:, :], cos_c[:, :])
                        nc.vector.tensor_mul(t2[:, :], sw[:, :], sin_c[:, :])
                        nc.vector.tensor_add(dst[:, tcol:tcol + CH], t1[:, :], t2[:, :])

            # ── phase 3: attention for one (batch, q-chunk), both heads
            # packed: the two heads' K=64 score matmuls run concurrently in
            # disjoint PE row-groups via tile_position, so all 128 array rows
            # stay active (HAM) and scores cost one matmul-time per pair.
            def emit_att2(b, qc):
                toff = b * S
                qcols = slice(toff + QW * qc, toff + QW * (qc + 1))
                ots = [pools["ot"].tile([HD + 1, QW], F32, name=f"ot{b}{h}{qc}", tag="ot")
                       for h in range(HPC)]

                def emit_scores(kt):
                    krows = slice(toff + 128 * kt, toff + 128 * (kt + 1))
                    exs = []
                    scs = []
                    for h in range(HPC):
                        po = HD * h
                        sc = pools["sc"].tile([128, QW], F32, name=f"sc{b}{h}{qc}{kt}", tag="sc")
                        nc.tensor.matmul(sc[:, :], ktr[po:po + HD, krows],
                                         qtr[po:po + HD, qcols], start=True, stop=True,
                                         tile_position=(po, 0))
                        scs.append(sc)
                    for h in range(HPC):
                        sc = scs[h]
                        if use_mask:
                            mtile = expp.tile([128, QW], F32, name=f"mt{b}{h}{qc}{kt}", tag="mt")
                            nc.sync.dma_start(
                                out=mtile,
                                in_=mt_d[128 * kt:128 * (kt + 1), QW * qc:QW * (qc + 1)])
                            nc.vector.tensor_scalar_mul(sc[:, :], sc[:, :], SCALE)
                            nc.vector.tensor_add(sc[:, :], sc[:, :], mtile[:, :])
                        ex = expp.tile([128, QW], BF16, name=f"ex{b}{h}{qc}{kt}", tag="ex")
                        nc.scalar.activation(ex[:, :], sc[:, :], AF.Exp,
                                             scale=(1.0 if use_mask else SCALE))
                        exs.append(ex)
                    return exs

                def emit_av(kt, exs):
                    kt_glob = (toff + 128 * kt) // 128
                    for h in range(HPC):
                        nc.tensor.matmul(ots[h][:, :],
                                         vnat[kt_glob][:, 65 * h:65 * h + HD + 1],
                                         exs[h][:, :], start=(kt == 0), stop=(kt == KT - 1))

                # software-pipelined: scores(kt+1) issue before AV(kt)
                exs = emit_scores(0)
                for kt in range(1, KT):
                    ex_next = emit_scores(kt)
                    emit_av(kt - 1, exs)
                    exs = ex_next
                emit_av(KT - 1, exs)
                # normalize: rows 0..63 divided by row 64
                for h in range(HPC):
                    po = HD * h
                    den = outp.tile([1, QW], F32, name=f"den{b}{h}{qc}", tag="den")
                    nc.vector.tensor_copy(den[:, :], ots[h][HD:HD + 1, :])
                    bcast = outp.tile([HD, QW], F32, name=f"bc{b}{h}{qc}", tag="bc")
                    nc.gpsimd.partition_broadcast(bcast[:, :], den[:, :])
                    rcp = outp.tile([HD, QW], F32, name=f"rcp{b}{h}{qc}", tag="rcp")
                    nc.vector.reciprocal_approx_fast(rcp[:, :], bcast[:, :])
                    nc.vector.tensor_mul(aoT[po:po + HD, qcols], ots[h][0:HD, :], rcp[:, :])

            # ── phase 4: partial output projection for one (batch,qchunk)
            def emit_oproj(b, qc):
                toff = b * S
                qcols = slice(toff + QW * qc, toff + QW * (qc + 1))
                for mo in range(D // 128):
                    yp = pools["proj"].tile([128, QW], F32, name=f"yp{b}{qc}{mo}", tag="proj")
                    nc.tensor.matmul(yp[:, :], wo[:, mo, :], aoT[:, qcols],
                                     start=True, stop=True)
                    yo = outp.tile([128, QW], F32, name=f"yo{b}{qc}{mo}", tag="yo")
                    nc.vector.tensor_copy(yo[:, :], yp[:, :])
                    nc.sync.dma_start(out=ypT_d[128 * mo:128 * (mo + 1), qcols], in_=yo)

            # ── emission: sequential phases, phase-scoped psum pools ────
            ps1 = ExitStack()
            ps_tp = ps1.enter_context(tc.tile_pool(name="ps_tp", bufs=4, space="PSUM"))
            ps_proj = ps1.enter_context(tc.tile_pool(name="ps_proj", bufs=3, space="PSUM"))
            pools = {"tp": ps_tp, "proj": ps_proj}
            for n in range(NCHUNK):
                emit_chunk(n)
            ps1.close()
            ps3 = ExitStack()
            pools["sc"] = ps3.enter_context(tc.tile_pool(name="ps_sc", bufs=3, space="PSUM"))
            pools["ot"] = ps3.enter_context(tc.tile_pool(name="ps_ot", bufs=3, space="PSUM"))
            pools["proj"] = ps3.enter_context(tc.tile_pool(name="ps_y", bufs=2, space="PSUM"))
            # O-proj lags one q-chunk behind attention so the PE never waits
            # on the normalize chain at group boundaries
            groups = [(b, qc) for b in range(B) for qc in range(QC)]
            prev = None
            for g in groups:
                emit_att2(*g)
                if prev is not None:
                    emit_oproj(*prev)
                prev = g
            emit_oproj(*prev)
            ps3.close()

    nc.compile()
    nc.m = get_hw_module(nc.m)
    return nc


def _get_nc(use_mask: bool):
    key = ("nc", use_mask)
    if key not in _CACHE:
        _CACHE[key] = _build(use_mask)
    return _CACHE[key]


def kernel(x, rope, mask, Wq, bq, Wk, bk, Wv, bv, Wo, bo, _trace=False):
    x = np.ascontiguousarray(np.asarray(x, dtype=np.float32))
    rope = np.asarray(rope, dtype=np.float32)
    mask = np.asarray(mask, dtype=np.float32)
    use_mask = bool(np.any(mask))

    x2d = x.reshape(T, D)
    cos = rope[0, 0, :, 0, :]                      # [S, 64]
    sin = rope[1, 0, :, 0, :]
    sgn = np.where(np.arange(HD) % 2 == 0, -1.0, 1.0).astype(np.float32)[:, None]
    cosT = np.ascontiguousarray(np.tile(cos.T, (HPC, B)))          # [128, T]
    sinT = np.ascontiguousarray(np.tile(sin.T * sgn, (HPC, B)))    # [128, T]
    psw = np.zeros((128, 128), dtype=np.float32)
    idx = np.arange(128)
    psw[idx ^ 1, idx] = 1.0

    nc = _get_nc(use_mask)
    in_maps = []
    for c in range(NC):
        cs = slice(CW * c, CW * (c + 1))
        m = dict(
            x=x2d,
            wq=np.ascontiguousarray(Wq[:, cs]), bq=np.ascontiguousarray(bq[cs]).reshape(CW, 1),
            wk=np.ascontiguousarray(Wk[:, cs]), bk=np.ascontiguousarray(bk[cs]).reshape(CW, 1),
            wv=np.ascontiguousarray(Wv[:, cs]), bv=np.ascontiguousarray(bv[cs]).reshape(CW, 1),
            wo=np.ascontiguousarray(Wo[cs, :]),
            cost=cosT, sint=sinT, pswap=psw,
        )
        if use_mask:
            m["maskt"] = np.ascontiguousarray(mask[0, 0].T)
        in_maps.append({k: np.asarray(v, dtype=np.float32) for k, v in m.items()})

    res = bass_utils.run_bass_kernel_spmd(
        nc, in_maps, core_ids=list(range(NC)), trace=_trace)
    # row-parallel unshard: sum the per-core partial projections, add bias
    ypT = res.results[0]["ypT"].astype(np.float32)
    for c in range(1, NC):
        ypT = ypT + res.results[c]["ypT"]
    out = (ypT.T + np.asarray(bo, dtype=np.float32)).reshape(B, S, D).astype(np.float32)
    out = np.ascontiguousarray(out)
    if _trace:
        return out, res
    return out


# revision 23
# speedup vs baseline: 1.3393x; 1.0170x over previous
"""Trainium2 Bass kernel for CosyVoice3 DiT attention (B=2, S=2048, H=16, hd=64, D=1024).

Sharding: tensor parallelism over heads — 2 heads per core on 8 cores.
Each core computes QKV projections for its head slice, RoPE, full attention
for its 2 heads, then its heads' contribution to the output projection
(row-parallel). The host gather sums the 8 partial outputs (the standard
row-parallel TP reduction) and adds the output bias.

Layout trick: everything is computed transposed ([dim, tokens]) so the
attention matmuls need no on-chip transposes of the big S x S matrices:
  scoresT[k,q] = K @ Q^T    (lhsT = K^T slice, rhs = Q^T slice)
  outT[d,q]    = V_aug^T @ expT  with V_aug = [V | ones] giving the softmax
                 denominator for free in row 64.
Softmax skips max-subtraction (scores are O(10) for this model family, and
exp is computed in fp32 which is safe up to ~88).

The emission order interleaves batch-0 attention with the tail of the
QKV-projection phase so the PE never idles long enough for the HAM clock
gate to re-throttle it to 1.2 GHz.
"""
import sys
sys.path.insert(0, "/opt/trn_rl_repo")
from contextlib import ExitStack
import numpy as np

# NTFF profile hook shim: this image's antenv lacks axon_hooks, which
# bass_utils imports unconditionally when trace=True (and the boot-time
# installer degrades silently without it). Recreate the module and install
# the ctypes-based hook so neuron-profile traces work.
import types as _types
try:
    import antenv as _antenv
    if "antenv.axon_hooks" not in sys.modules:
        _hooks = _types.ModuleType("antenv.axon_hooks")
        _hook_box = [None]
        _hooks.set_axon_ntff_profile_hook = lambda h: _hook_box.__setitem__(0, h)
        _hooks.get_axon_ntff_profile_hook = lambda: _hook_box[0]
        sys.modules["antenv.axon_hooks"] = _hooks
        _antenv.axon_hooks = _hooks
        try:
            from trn_agent_boot.trn_boot import _ntff_profile_via_ctypes
            _hooks.set_axon_ntff_profile_hook(
                _ntff_profile_via_ctypes("/opt/axon/libaxon_pjrt.so"))
        except Exception:
            pass
except Exception:
    pass

import concourse.bass as bass
import concourse.mybir as mybir
from concourse import bacc
from concourse.tile import TileContext
from concourse.bass_interp import get_hw_module
from concourse import bass_utils
from concourse.masks import make_identity
bass_utils.upload_artifacts = lambda tmpdir: str(tmpdir)  # no S3 in container

# ── constants (hardcoded per problem spec) ────────────────────────────────
B, S, D, H, HD = 2, 2048, 1024, 16, 64
T = B * S                 # 4096 tokens
NC = 8                    # cores
HPC = H // NC             # 2 heads per core
CW = HPC * HD             # 128 rows/cols per core
SCALE = 1.0 / np.sqrt(HD)
F32 = mybir.dt.float32
F32R = mybir.dt.float32r
BF16 = mybir.dt.bfloat16
AF = mybir.ActivationFunctionType

_CACHE = {}


def _build(use_mask: bool):
    nc = bacc.Bacc("TRN2", target_bir_lowering=False, debug=False, num_devices=NC)

    # inputs (per-core slices supplied by host)
    x_d = nc.dram_tensor("x", [T, D], F32, kind="ExternalInput")
    wq_d = nc.dram_tensor("wq", [D, CW], F32R, kind="ExternalInput")
    wk_d = nc.dram_tensor("wk", [D, CW], F32R, kind="ExternalInput")
    wv_d = nc.dram_tensor("wv", [D, CW], F32R, kind="ExternalInput")
    # wo: the CW rows of Wo owned by this core's heads -> [CW, D]
    wo_d = nc.dram_tensor("wo", [CW, D], F32R, kind="ExternalInput")
    bq_d = nc.dram_tensor("bq", [CW, 1], F32, kind="ExternalInput")
    bk_d = nc.dram_tensor("bk", [CW, 1], F32, kind="ExternalInput")
    bv_d = nc.dram_tensor("bv", [CW, 1], F32, kind="ExternalInput")
    cos_d = nc.dram_tensor("cost", [CW, T], F32, kind="ExternalInput")
    sin_d = nc.dram_tensor("sint", [CW, T], F32, kind="ExternalInput")   # sign-folded
    psw_d = nc.dram_tensor("pswap", [128, 128], F32R, kind="ExternalInput")
    if use_mask:
        mt_d = nc.dram_tensor("maskt", [S, S], F32, kind="ExternalInput")

    # partial output, transposed: ypT[n, t] = sum over this core's head dims
    ypT_d = nc.dram_tensor("ypT", [D, T], F32, kind="ExternalOutput")

    NCHUNK = 8            # token chunks of 512 for projections
    CH = T // NCHUNK      # 512
    KT = S // 128         # 16 k-tiles per batch
    QW = 512              # q chunk width
    QC = S // QW          # 4 q chunks per batch

    with TileContext(nc) as tc:
        with tc.tile_pool(name="persist", bufs=1) as persist, \
             tc.tile_pool(name="wpool", bufs=1) as wpool, \
             tc.tile_pool(name="xload", bufs=6) as xload, \
             tc.tile_pool(name="xtp", bufs=10) as xtpool, \
             tc.tile_pool(name="chunks", bufs=2) as chunks, \
             tc.tile_pool(name="expp", bufs=4) as expp, \
             tc.tile_pool(name="outp", bufs=3) as outp:

            # ── persistent tiles ────────────────────────────────────────
            ident = persist.tile([128, 128], F32, name="ident")
            make_identity(nc, ident)
            psw = persist.tile([128, 128], F32R, name="psw")
            nc.sync.dma_start(out=psw, in_=psw_d[:, :])
            wq = wpool.tile([128, D // 128, CW], F32R, name="wq_sb")
            wk = wpool.tile([128, D // 128, CW], F32R, name="wk_sb")
            wv = wpool.tile([128, D // 128, CW], F32R, name="wv_sb")
            for wt, wdr in ((wq, wq_d), (wk, wk_d), (wv, wv_d)):
                nc.sync.dma_start(out=wt, in_=wdr.ap().rearrange("(kc p) m -> p kc m", p=128))
            # wo rows for this core: [CW, D] -> lhsT chunks [CW, 128] per out-col group
            wo = wpool.tile([CW, D // 128, 128], F32R, name="wo_sb")
            nc.sync.dma_start(out=wo, in_=wo_d.ap().rearrange("p (mc m) -> p mc m", m=128))
            bq = wpool.tile([CW, 1], F32, name="bq_sb")
            bk = wpool.tile([CW, 1], F32, name="bk_sb")
            bv0 = wpool.tile([HD, 1], F32, name="bv0_sb")
            bv1 = wpool.tile([HD, 1], F32, name="bv1_sb")
            nc.sync.dma_start(out=bq, in_=bq_d[:, :])
            nc.sync.dma_start(out=bk, in_=bk_d[:, :])
            nc.sync.dma_start(out=bv0, in_=bv_d[0:HD, :])
            nc.sync.dma_start(out=bv1, in_=bv_d[HD:CW, :])

            qtr = persist.tile([128, T], BF16, name="qtr")    # rope'd Q^T
            ktr = persist.tile([128, T], BF16, name="ktr")    # rope'd K^T
            aoT = persist.tile([128, T], F32R, name="aoT")    # normalized attn out^T
            # V natural per k-tile: [128 tok, 2*(64+1)] with ones cols
            vnat = [persist.tile([128, 2 * (HD + 1)], BF16, name=f"vnat{i}")
                    for i in range(T // 128)]

            # ── phase 1: per token-chunk: transpose x, QKV proj, rope ───
            def emit_chunk(n):
                tcol = n * CH
                xts = [xtpool.tile([128, CH], F32R, name=f"xt{n}_{dc}", tag="xt")
                       for dc in range(D // 128)]
                xns = []
                for tt in range(CH // 128):
                    xn = xload.tile([128, D], F32, name=f"xn{n}_{tt}", tag="xn")
                    nc.sync.dma_start(out=xn, in_=x_d[tcol + 128 * tt: tcol + 128 * (tt + 1), :])
                    xns.append(xn)
                for dc in range(D // 128):
                    # pack the 4 token-block transposes of one d-block into one
                    # psum bank, one copy out
                    xp = pools["tp"].tile([128, CH], F32, name=f"xp{n}{dc}", tag="tp")
                    for tt in range(CH // 128):
                        nc.tensor.transpose(xp[:, 128 * tt:128 * (tt + 1)],
                                            xns[tt][:, 128 * dc:128 * (dc + 1)], ident)
                    nc.any.tensor_copy(xts[dc][:, :], xp[:, :])

                cos_c = chunks.tile([128, CH], F32, name=f"cos{n}", tag="cosc")
                sin_c = chunks.tile([128, CH], F32, name=f"sin{n}", tag="sinc")
                nc.sync.dma_start(out=cos_c, in_=cos_d[:, tcol:tcol + CH])
                nc.sync.dma_start(out=sin_c, in_=sin_d[:, tcol:tcol + CH])

                for name, wt, dst in (("q", wq, qtr), ("k", wk, ktr), ("v", wv, None)):
                    pp = pools["proj"].tile([128, CH], F32, name=f"{name}ps{n}", tag="proj")
                    for dc in range(D // 128):
                        nc.tensor.matmul(pp[:, :], wt[:, dc, :], xts[dc][:, :],
                                         start=(dc == 0), stop=(dc == D // 128 - 1))
                    if name == "v":
                        # per-head natural V via PE transpose; ones row becomes
                        # the denominator column after transpose
                        for h in range(HPC):
                            vth = chunks.tile([HD + 1, CH], F32, name=f"vt{n}{h}", tag="vth")
                            nc.scalar.activation(vth[0:HD, :], pp[HD * h:HD * (h + 1), :],
                                                 AF.Identity, bias=(bv0 if h == 0 else bv1))
                            nc.vector.memset(vth[HD:HD + 1, :], 1.0)
                            for ktl in range(CH // 128):
                                vp = pools["tp"].tile([128, HD + 1], F32, name=f"vp{n}{h}{ktl}", tag="tp")
                                nc.tensor.transpose(vp[:, :], vth[:, 128 * ktl:128 * (ktl + 1)],
                                                    ident[0:HD + 1, 0:HD + 1])
                                kt_glob = (tcol + 128 * ktl) // 128
                                nc.any.tensor_copy(
                                    vnat[kt_glob][:, 65 * h:65 * h + HD + 1], vp[:, :])
                    else:
                        # bias + rope: dst_chunk = (p+b)*cos + swap(p+b)*sin_signed
                        qb = chunks.tile([128, CH], F32R, name=f"{name}b{n}", tag="qb")
                        nc.scalar.activation(qb[:, :], pp[:, :], AF.Identity,
                                             bias=(bq if name == "q" else bk))
                        sw = pools["tp"].tile([128, CH], F32, name=f"{name}sw{n}", tag="tp")
                        for j in range(CH // 512):
                            nc.tensor.matmul(sw[:, 512 * j:512 * (j + 1)], psw,
                                             qb[:, 512 * j:512 * (j + 1)],
                                             start=True, stop=True)
                        t1 = chunks.tile([128, CH], F32, name=f"{name}t1{n}", tag="t1")
                        t2 = chunks.tile([128, CH], F32, name=f"{name}t2{n}", tag="t2")
                        nc.vector.tensor_mul(t1[:, :], qb[:, :], cos_c[:, :])
                        nc.vector.tensor_mul(t2[:, :], sw[:, :], sin_c[:, :])
                        nc.vector.tensor_add(dst[:, tcol:tcol + CH], t1[:, :], t2[:, :])

            # ── phase 3: attention for one (batch, q-chunk), both heads
            # packed: the two heads' K=64 score matmuls run concurrently in
            # disjoint PE row-groups via tile_position, so all 128 array rows
            # stay active (HAM) and scores cost one matmul-time per pair.
            def emit_att2(b, qc):
                toff = b * S
                qcols = slice(toff + QW * qc, toff + QW * (qc + 1))
                ots = [pools["ot"].tile([HD + 1, QW], F32, name=f"ot{b}{h}{qc}", tag="ot")
                       for h in range(HPC)]

                def emit_scores(kt):
                    krows = slice(toff + 128 * kt, toff + 128 * (kt + 1))
                    # both heads' scores side by side in one 2-bank psum tile;
                    # one exp instruction then covers both heads
                    sc = pools["sc"].tile([128, 2 * QW], F32, name=f"sc{b}{qc}{kt}", tag="sc")
                    for h in range(HPC):
                        po = HD * h
                        nc.tensor.matmul(sc[:, QW * h:QW * (h + 1)],
                                         ktr[po:po + HD, krows],
                                         qtr[po:po + HD, qcols], start=True, stop=True,
                                         tile_position=(po, 0))
                    if use_mask:
                        mtile = expp.tile([128, QW], F32, name=f"mt{b}{qc}{kt}", tag="mt")
                        nc.sync.dma_start(
                            out=mtile,
                            in_=mt_d[128 * kt:128 * (kt + 1), QW * qc:QW * (qc + 1)])
                        for h in range(HPC):
                            nc.vector.tensor_scalar_mul(
                                sc[:, QW * h:QW * (h + 1)], sc[:, QW * h:QW * (h + 1)], SCALE)
                            nc.vector.tensor_add(
                                sc[:, QW * h:QW * (h + 1)], sc[:, QW * h:QW * (h + 1)],
                                mtile[:, :])
                    ex = expp.tile([128, 2 * QW], BF16, name=f"ex{b}{qc}{kt}", tag="ex")
                    nc.scalar.activation(ex[:, :], sc[:, :], AF.Exp,
                                         scale=(1.0 if use_mask else SCALE))
                    return ex

                def emit_av(kt, ex):
                    kt_glob = (toff + 128 * kt) // 128
                    for h in range(HPC):
                        nc.tensor.matmul(ots[h][:, :],
                                         vnat[kt_glob][:, 65 * h:65 * h + HD + 1],
                                         ex[:, QW * h:QW * (h + 1)],
                                         start=(kt == 0), stop=(kt == KT - 1))

                # software-pipelined: scores(kt+1) issue before AV(kt)
                exs = emit_scores(0)
                for kt in range(1, KT):
                    ex_next = emit_scores(kt)
                    emit_av(kt - 1, exs)
                    exs = ex_next
                emit_av(KT - 1, exs)
                # normalize: rows 0..63 divided by row 64
                for h in range(HPC):
                    po = HD * h
                    den = outp.tile([1, QW], F32, name=f"den{b}{h}{qc}", tag="den")
                    nc.vector.tensor_copy(den[:, :], ots[h][HD:HD + 1, :])
                    bcast = outp.tile([HD, QW], F32, name=f"bc{b}{h}{qc}", tag="bc")
                    nc.gpsimd.partition_broadcast(bcast[:, :], den[:, :])
                    rcp = outp.tile([HD, QW], F32, name=f"rcp{b}{h}{qc}", tag="rcp")
                    nc.vector.reciprocal_approx_fast(rcp[:, :], bcast[:, :])
                    nc.vector.tensor_mul(aoT[po:po + HD, qcols], ots[h][0:HD, :], rcp[:, :])

            # ── phase 4: partial output projection for one (batch,qchunk)
            def emit_oproj(b, qc):
                toff = b * S
                qcols = slice(toff + QW * qc, toff + QW * (qc + 1))
                for mo in range(D // 128):
                    yp = pools["proj"].tile([128, QW], F32, name=f"yp{b}{qc}{mo}", tag="sc")
                    nc.tensor.matmul(yp[:, :], wo[:, mo, :], aoT[:, qcols],
                                     start=True, stop=True)
                    yo = outp.tile([128, QW], F32, name=f"yo{b}{qc}{mo}", tag="yo")
                    nc.vector.tensor_copy(yo[:, :], yp[:, :])
                    nc.sync.dma_start(out=ypT_d[128 * mo:128 * (mo + 1), qcols], in_=yo)

            # ── emission: sequential phases, phase-scoped psum pools ────
            ps1 = ExitStack()
            ps_tp = ps1.enter_context(tc.tile_pool(name="ps_tp", bufs=4, space="PSUM"))
            ps_proj = ps1.enter_context(tc.tile_pool(name="ps_proj", bufs=3, space="PSUM"))
            pools = {"tp": ps_tp, "proj": ps_proj}
            for n in range(NCHUNK):
                emit_chunk(n)
            ps1.close()
            ps3 = ExitStack()
            pools["sc"] = ps3.enter_context(tc.tile_pool(name="ps_sc", bufs=3, space="PSUM"))
            pools["ot"] = ps3.enter_context(tc.tile_pool(name="ps_ot", bufs=2, space="PSUM"))
            pools["proj"] = pools["sc"]
            # O-proj lags one q-chunk behind attention so the PE never waits
            # on the normalize chain at group boundaries
            groups = [(b, qc) for b in range(B) for qc in range(QC)]
            prev = None
            for g in groups:
                emit_att2(*g)
                if prev is not None:
                    emit_oproj(*prev)
                prev = g
            emit_oproj(*prev)
            ps3.close()

    nc.compile()
    nc.m = get_hw_module(nc.m)
    return nc


def _get_nc(use_mask: bool):
    key = ("nc", use_mask)
    if key not in _CACHE:
        _CACHE[key] = _build(use_mask)
    return _CACHE[key]


def kernel(x, rope, mask, Wq, bq, Wk, bk, Wv, bv, Wo, bo, _trace=False):
    x = np.ascontiguousarray(np.asarray(x, dtype=np.float32))
    rope = np.asarray(rope, dtype=np.float32)
    mask = np.asarray(mask, dtype=np.float32)
    use_mask = bool(np.any(mask))

    x2d = x.reshape(T, D)
    cos = rope[0, 0, :, 0, :]                      # [S, 64]
    sin = rope[1, 0, :, 0, :]
    sgn = np.where(np.arange(HD) % 2 == 0, -1.0, 1.0).astype(np.float32)[:, None]
    cosT = np.ascontiguousarray(np.tile(cos.T, (HPC, B)))          # [128, T]
    sinT = np.ascontiguousarray(np.tile(sin.T * sgn, (HPC, B)))    # [128, T]
    psw = np.zeros((128, 128), dtype=np.float32)
    idx = np.arange(128)
    psw[idx ^ 1, idx] = 1.0

    nc = _get_nc(use_mask)
    in_maps = []
    for c in range(NC):
        cs = slice(CW * c, CW * (c + 1))
        m = dict(
            x=x2d,
            wq=np.ascontiguousarray(Wq[:, cs]), bq=np.ascontiguousarray(bq[cs]).reshape(CW, 1),
            wk=np.ascontiguousarray(Wk[:, cs]), bk=np.ascontiguousarray(bk[cs]).reshape(CW, 1),
            wv=np.ascontiguousarray(Wv[:, cs]), bv=np.ascontiguousarray(bv[cs]).reshape(CW, 1),
            wo=np.ascontiguousarray(Wo[cs, :]),
            cost=cosT, sint=sinT, pswap=psw,
        )
        if use_mask:
            m["maskt"] = np.ascontiguousarray(mask[0, 0].T)
        in_maps.append({k: np.asarray(v, dtype=np.float32) for k, v in m.items()})

    res = bass_utils.run_bass_kernel_spmd(
        nc, in_maps, core_ids=list(range(NC)), trace=_trace)
    # row-parallel unshard: sum the per-core partial projections, add bias
    ypT = res.results[0]["ypT"].astype(np.float32)
    for c in range(1, NC):
        ypT = ypT + res.results[c]["ypT"]
    out = (ypT.T + np.asarray(bo, dtype=np.float32)).reshape(B, S, D).astype(np.float32)
    out = np.ascontiguousarray(out)
    if _trace:
        return out, res
    return out


# revision 24
# speedup vs baseline: 1.3743x; 1.0261x over previous
"""Trainium2 Bass kernel for CosyVoice3 DiT attention (B=2, S=2048, H=16, hd=64, D=1024).

Sharding: tensor parallelism over heads — 2 heads per core on 8 cores.
Each core computes QKV projections for its head slice, RoPE, full attention
for its 2 heads, then its heads' contribution to the output projection
(row-parallel). The host gather sums the 8 partial outputs (the standard
row-parallel TP reduction) and adds the output bias.

Layout trick: everything is computed transposed ([dim, tokens]) so the
attention matmuls need no on-chip transposes of the big S x S matrices:
  scoresT[k,q] = K @ Q^T    (lhsT = K^T slice, rhs = Q^T slice)
  outT[d,q]    = V_aug^T @ expT  with V_aug = [V | ones] giving the softmax
                 denominator for free in row 64.
Softmax skips max-subtraction (scores are O(10) for this model family, and
exp is computed in fp32 which is safe up to ~88).

The emission order interleaves batch-0 attention with the tail of the
QKV-projection phase so the PE never idles long enough for the HAM clock
gate to re-throttle it to 1.2 GHz.
"""
import sys
sys.path.insert(0, "/opt/trn_rl_repo")
from contextlib import ExitStack
import numpy as np

# NTFF profile hook shim: this image's antenv lacks axon_hooks, which
# bass_utils imports unconditionally when trace=True (and the boot-time
# installer degrades silently without it). Recreate the module and install
# the ctypes-based hook so neuron-profile traces work.
import types as _types
try:
    import antenv as _antenv
    if "antenv.axon_hooks" not in sys.modules:
        _hooks = _types.ModuleType("antenv.axon_hooks")
        _hook_box = [None]
        _hooks.set_axon_ntff_profile_hook = lambda h: _hook_box.__setitem__(0, h)
        _hooks.get_axon_ntff_profile_hook = lambda: _hook_box[0]
        sys.modules["antenv.axon_hooks"] = _hooks
        _antenv.axon_hooks = _hooks
        try:
            from trn_agent_boot.trn_boot import _ntff_profile_via_ctypes
            _hooks.set_axon_ntff_profile_hook(
                _ntff_profile_via_ctypes("/opt/axon/libaxon_pjrt.so"))
        except Exception:
            pass
except Exception:
    pass

import concourse.bass as bass
import concourse.mybir as mybir
from concourse import bacc
from concourse.tile import TileContext
from concourse.bass_interp import get_hw_module
from concourse import bass_utils
from concourse.masks import make_identity
bass_utils.upload_artifacts = lambda tmpdir: str(tmpdir)  # no S3 in container

# ── constants (hardcoded per problem spec) ────────────────────────────────
B, S, D, H, HD = 2, 2048, 1024, 16, 64
T = B * S                 # 4096 tokens
NC = 8                    # cores
HPC = H // NC             # 2 heads per core
CW = HPC * HD             # 128 rows/cols per core
SCALE = 1.0 / np.sqrt(HD)
F32 = mybir.dt.float32
F32R = mybir.dt.float32r
BF16 = mybir.dt.bfloat16
AF = mybir.ActivationFunctionType

_CACHE = {}


def _build(use_mask: bool):
    nc = bacc.Bacc("TRN2", target_bir_lowering=False, debug=False, num_devices=NC)

    # inputs (per-core slices supplied by host)
    x_d = nc.dram_tensor("x", [T, D], F32, kind="ExternalInput")
    wq_d = nc.dram_tensor("wq", [D, CW], F32R, kind="ExternalInput")
    wk_d = nc.dram_tensor("wk", [D, CW], F32R, kind="ExternalInput")
    wv_d = nc.dram_tensor("wv", [D, CW], F32R, kind="ExternalInput")
    # wo: the CW rows of Wo owned by this core's heads -> [CW, D]
    wo_d = nc.dram_tensor("wo", [CW, D], F32R, kind="ExternalInput")
    bq_d = nc.dram_tensor("bq", [CW, 1], F32, kind="ExternalInput")
    bk_d = nc.dram_tensor("bk", [CW, 1], F32, kind="ExternalInput")
    bv_d = nc.dram_tensor("bv", [CW, 1], F32, kind="ExternalInput")
    cos_d = nc.dram_tensor("cost", [CW, T], F32, kind="ExternalInput")
    sin_d = nc.dram_tensor("sint", [CW, T], F32, kind="ExternalInput")   # sign-folded
    psw_d = nc.dram_tensor("pswap", [128, 128], F32R, kind="ExternalInput")
    if use_mask:
        mt_d = nc.dram_tensor("maskt", [S, S], F32, kind="ExternalInput")

    # partial output, transposed: ypT[n, t] = sum over this core's head dims
    ypT_d = nc.dram_tensor("ypT", [D, T], F32, kind="ExternalOutput")

    NCHUNK = 8            # token chunks of 512 for projections
    CH = T // NCHUNK      # 512
    KT = S // 128         # 16 k-tiles per batch
    QW = 512              # q chunk width
    QC = S // QW          # 4 q chunks per batch

    with TileContext(nc) as tc:
        with tc.tile_pool(name="persist", bufs=1) as persist, \
             tc.tile_pool(name="wpool", bufs=1) as wpool, \
             tc.tile_pool(name="xload", bufs=6) as xload, \
             tc.tile_pool(name="xtp", bufs=12) as xtpool, \
             tc.tile_pool(name="chunks", bufs=3) as chunks, \
             tc.tile_pool(name="expp", bufs=6) as expp, \
             tc.tile_pool(name="outp", bufs=4) as outp:

            # ── persistent tiles ────────────────────────────────────────
            ident = persist.tile([128, 128], F32, name="ident")
            make_identity(nc, ident)
            psw = persist.tile([128, 128], F32R, name="psw")
            nc.sync.dma_start(out=psw, in_=psw_d[:, :])
            wq = wpool.tile([128, D // 128, CW], F32R, name="wq_sb")
            wk = wpool.tile([128, D // 128, CW], F32R, name="wk_sb")
            wv = wpool.tile([128, D // 128, CW], F32R, name="wv_sb")
            for wt, wdr in ((wq, wq_d), (wk, wk_d), (wv, wv_d)):
                nc.sync.dma_start(out=wt, in_=wdr.ap().rearrange("(kc p) m -> p kc m", p=128))
            # wo rows for this core: [CW, D] -> lhsT chunks [CW, 128] per out-col group
            wo = wpool.tile([CW, D // 128, 128], F32R, name="wo_sb")
            nc.sync.dma_start(out=wo, in_=wo_d.ap().rearrange("p (mc m) -> p mc m", m=128))
            bq = wpool.tile([CW, 1], F32, name="bq_sb")
            bk = wpool.tile([CW, 1], F32, name="bk_sb")
            bv0 = wpool.tile([HD, 1], F32, name="bv0_sb")
            bv1 = wpool.tile([HD, 1], F32, name="bv1_sb")
            nc.sync.dma_start(out=bq, in_=bq_d[:, :])
            nc.sync.dma_start(out=bk, in_=bk_d[:, :])
            nc.sync.dma_start(out=bv0, in_=bv_d[0:HD, :])
            nc.sync.dma_start(out=bv1, in_=bv_d[HD:CW, :])

            qtr = persist.tile([128, T], BF16, name="qtr")    # rope'd Q^T
            ktr = persist.tile([128, T], BF16, name="ktr")    # rope'd K^T
            aoT = persist.tile([128, T], F32R, name="aoT")    # normalized attn out^T
            # V natural per k-tile: [128 tok, 2*(64+1)] with ones cols
            vnat = [persist.tile([128, 2 * (HD + 1)], BF16, name=f"vnat{i}")
                    for i in range(T // 128)]

            # ── phase 1: per token-chunk: transpose x, QKV proj, rope ───
            def emit_chunk(n):
                tcol = n * CH
                xts = [xtpool.tile([128, CH], F32R, name=f"xt{n}_{dc}", tag="xt")
                       for dc in range(D // 128)]
                xns = []
                for tt in range(CH // 128):
                    xn = xload.tile([128, D], F32, name=f"xn{n}_{tt}", tag="xn")
                    nc.sync.dma_start(out=xn, in_=x_d[tcol + 128 * tt: tcol + 128 * (tt + 1), :])
                    xns.append(xn)
                for dc in range(D // 128):
                    # pack the 4 token-block transposes of one d-block into one
                    # psum bank, one copy out
                    xp = pools["tp"].tile([128, CH], F32, name=f"xp{n}{dc}", tag="tp")
                    for tt in range(CH // 128):
                        nc.tensor.transpose(xp[:, 128 * tt:128 * (tt + 1)],
                                            xns[tt][:, 128 * dc:128 * (dc + 1)], ident)
                    nc.any.tensor_copy(xts[dc][:, :], xp[:, :])

                cos_c = chunks.tile([128, CH], F32, name=f"cos{n}", tag="cosc")
                sin_c = chunks.tile([128, CH], F32, name=f"sin{n}", tag="sinc")
                nc.sync.dma_start(out=cos_c, in_=cos_d[:, tcol:tcol + CH])
                nc.sync.dma_start(out=sin_c, in_=sin_d[:, tcol:tcol + CH])

                for name, wt, dst in (("q", wq, qtr), ("k", wk, ktr), ("v", wv, None)):
                    pp = pools["proj"].tile([128, CH], F32, name=f"{name}ps{n}", tag="proj")
                    for dc in range(D // 128):
                        nc.tensor.matmul(pp[:, :], wt[:, dc, :], xts[dc][:, :],
                                         start=(dc == 0), stop=(dc == D // 128 - 1))
                    if name == "v":
                        # per-head natural V via PE transpose; ones row becomes
                        # the denominator column after transpose
                        for h in range(HPC):
                            vth = chunks.tile([HD + 1, CH], F32, name=f"vt{n}{h}", tag="vth")
                            nc.scalar.activation(vth[0:HD, :], pp[HD * h:HD * (h + 1), :],
                                                 AF.Identity, bias=(bv0 if h == 0 else bv1))
                            nc.vector.memset(vth[HD:HD + 1, :], 1.0)
                            for ktl in range(CH // 128):
                                vp = pools["tp"].tile([128, HD + 1], F32, name=f"vp{n}{h}{ktl}", tag="tp")
                                nc.tensor.transpose(vp[:, :], vth[:, 128 * ktl:128 * (ktl + 1)],
                                                    ident[0:HD + 1, 0:HD + 1])
                                kt_glob = (tcol + 128 * ktl) // 128
                                nc.any.tensor_copy(
                                    vnat[kt_glob][:, 65 * h:65 * h + HD + 1], vp[:, :])
                    else:
                        # bias + rope: dst_chunk = (p+b)*cos + swap(p+b)*sin_signed
                        qb = chunks.tile([128, CH], F32R, name=f"{name}b{n}", tag="qb")
                        nc.scalar.activation(qb[:, :], pp[:, :], AF.Identity,
                                             bias=(bq if name == "q" else bk))
                        sw = pools["tp"].tile([128, CH], F32, name=f"{name}sw{n}", tag="tp")
                        for j in range(CH // 512):
                            nc.tensor.matmul(sw[:, 512 * j:512 * (j + 1)], psw,
                                             qb[:, 512 * j:512 * (j + 1)],
                                             start=True, stop=True)
                        t1 = chunks.tile([128, CH], F32, name=f"{name}t1{n}", tag="t1")
                        t2 = chunks.tile([128, CH], F32, name=f"{name}t2{n}", tag="t2")
                        nc.vector.tensor_mul(t1[:, :], qb[:, :], cos_c[:, :])
                        nc.vector.tensor_mul(t2[:, :], sw[:, :], sin_c[:, :])
                        nc.vector.tensor_add(dst[:, tcol:tcol + CH], t1[:, :], t2[:, :])

            # ── phase 3: attention for one (batch, q-chunk), both heads
            # packed: the two heads' K=64 score matmuls run concurrently in
            # disjoint PE row-groups via tile_position, so all 128 array rows
            # stay active (HAM) and scores cost one matmul-time per pair.
            def emit_att2(b, qc):
                toff = b * S
                qcols = slice(toff + QW * qc, toff + QW * (qc + 1))
                ots = [pools["ot"].tile([HD + 1, QW], F32, name=f"ot{b}{h}{qc}", tag="ot")
                       for h in range(HPC)]

                def emit_scores(kt):
                    krows = slice(toff + 128 * kt, toff + 128 * (kt + 1))
                    # both heads' scores side by side in one 2-bank psum tile;
                    # one exp instruction then covers both heads
                    sc = pools["sc"].tile([128, 2 * QW], F32, name=f"sc{b}{qc}{kt}", tag="sc")
                    for h in range(HPC):
                        po = HD * h
                        nc.tensor.matmul(sc[:, QW * h:QW * (h + 1)],
                                         ktr[po:po + HD, krows],
                                         qtr[po:po + HD, qcols], start=True, stop=True,
                                         tile_position=(po, 0))
                    if use_mask:
                        mtile = expp.tile([128, QW], F32, name=f"mt{b}{qc}{kt}", tag="mt")
                        nc.sync.dma_start(
                            out=mtile,
                            in_=mt_d[128 * kt:128 * (kt + 1), QW * qc:QW * (qc + 1)])
                        for h in range(HPC):
                            nc.vector.tensor_scalar_mul(
                                sc[:, QW * h:QW * (h + 1)], sc[:, QW * h:QW * (h + 1)], SCALE)
                            nc.vector.tensor_add(
                                sc[:, QW * h:QW * (h + 1)], sc[:, QW * h:QW * (h + 1)],
                                mtile[:, :])
                    ex = expp.tile([128, 2 * QW], BF16, name=f"ex{b}{qc}{kt}", tag="ex")
                    nc.scalar.activation(ex[:, :], sc[:, :], AF.Exp,
                                         scale=(1.0 if use_mask else SCALE))
                    return ex

                def emit_av(kt, ex):
                    kt_glob = (toff + 128 * kt) // 128
                    for h in range(HPC):
                        nc.tensor.matmul(ots[h][:, :],
                                         vnat[kt_glob][:, 65 * h:65 * h + HD + 1],
                                         ex[:, QW * h:QW * (h + 1)],
                                         start=(kt == 0), stop=(kt == KT - 1))

                # software-pipelined: scores(kt+1) issue before AV(kt)
                exs = emit_scores(0)
                for kt in range(1, KT):
                    ex_next = emit_scores(kt)
                    emit_av(kt - 1, exs)
                    exs = ex_next
                emit_av(KT - 1, exs)
                # normalize: rows 0..63 divided by row 64
                for h in range(HPC):
                    po = HD * h
                    den = outp.tile([1, QW], F32, name=f"den{b}{h}{qc}", tag="den")
                    nc.vector.tensor_copy(den[:, :], ots[h][HD:HD + 1, :])
                    bcast = outp.tile([HD, QW], F32, name=f"bc{b}{h}{qc}", tag="bc")
                    nc.gpsimd.partition_broadcast(bcast[:, :], den[:, :])
                    rcp = outp.tile([HD, QW], F32, name=f"rcp{b}{h}{qc}", tag="rcp")
                    nc.vector.reciprocal_approx_fast(rcp[:, :], bcast[:, :])
                    nc.vector.tensor_mul(aoT[po:po + HD, qcols], ots[h][0:HD, :], rcp[:, :])

            # ── phase 4: partial output projection for one (batch,qchunk)
            def emit_oproj(b, qc):
                toff = b * S
                qcols = slice(toff + QW * qc, toff + QW * (qc + 1))
                for mo in range(D // 128):
                    yp = pools["proj"].tile([128, QW], F32, name=f"yp{b}{qc}{mo}", tag="sc")
                    nc.tensor.matmul(yp[:, :], wo[:, mo, :], aoT[:, qcols],
                                     start=True, stop=True)
                    yo = outp.tile([128, QW], F32, name=f"yo{b}{qc}{mo}", tag="yo")
                    nc.vector.tensor_copy(yo[:, :], yp[:, :])
                    nc.sync.dma_start(out=ypT_d[128 * mo:128 * (mo + 1), qcols], in_=yo)

            # ── emission: sequential phases, phase-scoped psum pools ────
            ps1 = ExitStack()
            ps_tp = ps1.enter_context(tc.tile_pool(name="ps_tp", bufs=4, space="PSUM"))
            ps_proj = ps1.enter_context(tc.tile_pool(name="ps_proj", bufs=3, space="PSUM"))
            pools = {"tp": ps_tp, "proj": ps_proj}
            for n in range(NCHUNK):
                emit_chunk(n)
            ps1.close()
            ps3 = ExitStack()
            pools["sc"] = ps3.enter_context(tc.tile_pool(name="ps_sc", bufs=3, space="PSUM"))
            pools["ot"] = ps3.enter_context(tc.tile_pool(name="ps_ot", bufs=2, space="PSUM"))
            pools["proj"] = pools["sc"]
            # O-proj lags one q-chunk behind attention so the PE never waits
            # on the normalize chain at group boundaries
            groups = [(b, qc) for b in range(B) for qc in range(QC)]
            prev = None
            for g in groups:
                emit_att2(*g)
                if prev is not None:
                    emit_oproj(*prev)
                prev = g
            emit_oproj(*prev)
            ps3.close()

    nc.compile()
    nc.m = get_hw_module(nc.m)
    return nc


def _get_nc(use_mask: bool):
    key = ("nc", use_mask)
    if key not in _CACHE:
        _CACHE[key] = _build(use_mask)
    return _CACHE[key]


def kernel(x, rope, mask, Wq, bq, Wk, bk, Wv, bv, Wo, bo, _trace=False):
    x = np.ascontiguousarray(np.asarray(x, dtype=np.float32))
    rope = np.asarray(rope, dtype=np.float32)
    mask = np.asarray(mask, dtype=np.float32)
    use_mask = bool(np.any(mask))

    x2d = x.reshape(T, D)
    cos = rope[0, 0, :, 0, :]                      # [S, 64]
    sin = rope[1, 0, :, 0, :]
    sgn = np.where(np.arange(HD) % 2 == 0, -1.0, 1.0).astype(np.float32)[:, None]
    cosT = np.ascontiguousarray(np.tile(cos.T, (HPC, B)))          # [128, T]
    sinT = np.ascontiguousarray(np.tile(sin.T * sgn, (HPC, B)))    # [128, T]
    psw = np.zeros((128, 128), dtype=np.float32)
    idx = np.arange(128)
    psw[idx ^ 1, idx] = 1.0

    nc = _get_nc(use_mask)
    in_maps = []
    for c in range(NC):
        cs = slice(CW * c, CW * (c + 1))
        m = dict(
            x=x2d,
            wq=np.ascontiguousarray(Wq[:, cs]), bq=np.ascontiguousarray(bq[cs]).reshape(CW, 1),
            wk=np.ascontiguousarray(Wk[:, cs]), bk=np.ascontiguousarray(bk[cs]).reshape(CW, 1),
            wv=np.ascontiguousarray(Wv[:, cs]), bv=np.ascontiguousarray(bv[cs]).reshape(CW, 1),
            wo=np.ascontiguousarray(Wo[cs, :]),
            cost=cosT, sint=sinT, pswap=psw,
        )
        if use_mask:
            m["maskt"] = np.ascontiguousarray(mask[0, 0].T)
        in_maps.append({k: np.asarray(v, dtype=np.float32) for k, v in m.items()})

    res = bass_utils.run_bass_kernel_spmd(
        nc, in_maps, core_ids=list(range(NC)), trace=_trace)
    # row-parallel unshard: sum the per-core partial projections, add bias
    ypT = res.results[0]["ypT"].astype(np.float32)
    for c in range(1, NC):
        ypT = ypT + res.results[c]["ypT"]
    out = (ypT.T + np.asarray(bo, dtype=np.float32)).reshape(B, S, D).astype(np.float32)
    out = np.ascontiguousarray(out)
    if _trace:
        return out, res
    return out


# revision 25
# speedup vs baseline: 1.3743x; 1.0000x over previous
"""Trainium2 Bass kernel for CosyVoice3 DiT attention (B=2, S=2048, H=16, hd=64, D=1024).

Sharding: tensor parallelism over heads — 2 heads per core on 8 cores.
Each core computes QKV projections for its head slice, RoPE, full attention
for its 2 heads, then its heads' contribution to the output projection
(row-parallel). The host gather sums the 8 partial outputs (the standard
row-parallel TP reduction) and adds the output bias.

Layout trick: everything is computed transposed ([dim, tokens]) so the
attention matmuls need no on-chip transposes of the big S x S matrices:
  scoresT[k,q] = K @ Q^T    (lhsT = K^T slice, rhs = Q^T slice)
  outT[d,q]    = V_aug^T @ expT  with V_aug = [V | ones] giving the softmax
                 denominator for free in row 64.
Softmax skips max-subtraction (scores are O(10) for this model family, and
exp is computed in fp32 which is safe up to ~88).

The emission order interleaves batch-0 attention with the tail of the
QKV-projection phase so the PE never idles long enough for the HAM clock
gate to re-throttle it to 1.2 GHz.
"""
import sys
sys.path.insert(0, "/opt/trn_rl_repo")
from contextlib import ExitStack
import numpy as np

# NTFF profile hook shim: this image's antenv lacks axon_hooks, which
# bass_utils imports unconditionally when trace=True (and the boot-time
# installer degrades silently without it). Recreate the module and install
# the ctypes-based hook so neuron-profile traces work.
import types as _types
try:
    import antenv as _antenv
    if "antenv.axon_hooks" not in sys.modules:
        _hooks = _types.ModuleType("antenv.axon_hooks")
        _hook_box = [None]
        _hooks.set_axon_ntff_profile_hook = lambda h: _hook_box.__setitem__(0, h)
        _hooks.get_axon_ntff_profile_hook = lambda: _hook_box[0]
        sys.modules["antenv.axon_hooks"] = _hooks
        _antenv.axon_hooks = _hooks
        try:
            from trn_agent_boot.trn_boot import _ntff_profile_via_ctypes
            _hooks.set_axon_ntff_profile_hook(
                _ntff_profile_via_ctypes("/opt/axon/libaxon_pjrt.so"))
        except Exception:
            pass
except Exception:
    pass

import concourse.bass as bass
import concourse.mybir as mybir
from concourse import bacc
from concourse.tile import TileContext
from concourse.bass_interp import get_hw_module
from concourse import bass_utils
from concourse.masks import make_identity
bass_utils.upload_artifacts = lambda tmpdir: str(tmpdir)  # no S3 in container

# ── constants (hardcoded per problem spec) ────────────────────────────────
B, S, D, H, HD = 2, 2048, 1024, 16, 64
T = B * S                 # 4096 tokens
NC = 8                    # cores
HPC = H // NC             # 2 heads per core
CW = HPC * HD             # 128 rows/cols per core
SCALE = 1.0 / np.sqrt(HD)
F32 = mybir.dt.float32
F32R = mybir.dt.float32r
BF16 = mybir.dt.bfloat16
AF = mybir.ActivationFunctionType

_CACHE = {}


def _build(use_mask: bool):
    nc = bacc.Bacc("TRN2", target_bir_lowering=False, debug=False, num_devices=NC)

    # inputs (per-core slices supplied by host)
    x_d = nc.dram_tensor("x", [T, D], F32, kind="ExternalInput")
    wq_d = nc.dram_tensor("wq", [D, CW], F32R, kind="ExternalInput")
    wk_d = nc.dram_tensor("wk", [D, CW], F32R, kind="ExternalInput")
    wv_d = nc.dram_tensor("wv", [D, CW], F32R, kind="ExternalInput")
    # wo: the CW rows of Wo owned by this core's heads -> [CW, D]
    wo_d = nc.dram_tensor("wo", [CW, D], F32R, kind="ExternalInput")
    bq_d = nc.dram_tensor("bq", [CW, 1], F32, kind="ExternalInput")
    bk_d = nc.dram_tensor("bk", [CW, 1], F32, kind="ExternalInput")
    bv_d = nc.dram_tensor("bv", [CW, 1], F32, kind="ExternalInput")
    cos_d = nc.dram_tensor("cost", [CW, T], F32, kind="ExternalInput")
    sin_d = nc.dram_tensor("sint", [CW, T], F32, kind="ExternalInput")   # sign-folded
    psw_d = nc.dram_tensor("pswap", [128, 128], F32R, kind="ExternalInput")
    if use_mask:
        mt_d = nc.dram_tensor("maskt", [S, S], F32, kind="ExternalInput")

    # partial output, transposed: ypT[n, t] = sum over this core's head dims
    ypT_d = nc.dram_tensor("ypT", [D, T], F32, kind="ExternalOutput")

    NCHUNK = 8            # token chunks of 512 for projections
    CH = T // NCHUNK      # 512
    KT = S // 128         # 16 k-tiles per batch
    QW = 512              # q chunk width
    QC = S // QW          # 4 q chunks per batch

    with TileContext(nc) as tc:
        with tc.tile_pool(name="persist", bufs=1) as persist, \
             tc.tile_pool(name="wpool", bufs=1) as wpool, \
             tc.tile_pool(name="xload", bufs=6) as xload, \
             tc.tile_pool(name="xtp", bufs=12) as xtpool, \
             tc.tile_pool(name="chunks", bufs=3) as chunks, \
             tc.tile_pool(name="expp", bufs=6) as expp, \
             tc.tile_pool(name="outp", bufs=4) as outp:

            # ── persistent tiles ────────────────────────────────────────
            ident = persist.tile([128, 128], F32, name="ident")
            make_identity(nc, ident)
            psw = persist.tile([128, 128], F32R, name="psw")
            nc.sync.dma_start(out=psw, in_=psw_d[:, :])
            wq = wpool.tile([128, D // 128, CW], F32R, name="wq_sb")
            wk = wpool.tile([128, D // 128, CW], F32R, name="wk_sb")
            wv = wpool.tile([128, D // 128, CW], F32R, name="wv_sb")
            for wt, wdr in ((wq, wq_d), (wk, wk_d), (wv, wv_d)):
                nc.sync.dma_start(out=wt, in_=wdr.ap().rearrange("(kc p) m -> p kc m", p=128))
            # wo rows for this core: [CW, D] -> lhsT chunks [CW, 128] per out-col group
            wo = wpool.tile([CW, D // 128, 128], F32R, name="wo_sb")
            nc.sync.dma_start(out=wo, in_=wo_d.ap().rearrange("p (mc m) -> p mc m", m=128))
            bq = wpool.tile([CW, 1], F32, name="bq_sb")
            bk = wpool.tile([CW, 1], F32, name="bk_sb")
            bv0 = wpool.tile([HD, 1], F32, name="bv0_sb")
            bv1 = wpool.tile([HD, 1], F32, name="bv1_sb")
            nc.sync.dma_start(out=bq, in_=bq_d[:, :])
            nc.sync.dma_start(out=bk, in_=bk_d[:, :])
            nc.sync.dma_start(out=bv0, in_=bv_d[0:HD, :])
            nc.sync.dma_start(out=bv1, in_=bv_d[HD:CW, :])

            qtr = persist.tile([128, T], BF16, name="qtr")    # rope'd Q^T
            ktr = persist.tile([128, T], BF16, name="ktr")    # rope'd K^T
            aoT = persist.tile([128, T], F32R, name="aoT")    # normalized attn out^T
            # V natural per k-tile: [128 tok, 2*(64+1)] with ones cols
            vnat = [persist.tile([128, 2 * (HD + 1)], BF16, name=f"vnat{i}")
                    for i in range(T // 128)]

            # ── phase 1: per token-chunk: transpose x, QKV proj, rope ───
            def emit_chunk(n):
                tcol = n * CH
                xts = [xtpool.tile([128, CH], F32R, name=f"xt{n}_{dc}", tag="xt")
                       for dc in range(D // 128)]
                xns = []
                for tt in range(CH // 128):
                    xn = xload.tile([128, D], F32, name=f"xn{n}_{tt}", tag="xn")
                    nc.sync.dma_start(out=xn, in_=x_d[tcol + 128 * tt: tcol + 128 * (tt + 1), :])
                    xns.append(xn)
                for dc in range(D // 128):
                    # pack the 4 token-block transposes of one d-block into one
                    # psum bank, one copy out
                    xp = pools["tp"].tile([128, CH], F32, name=f"xp{n}{dc}", tag="tp")
                    for tt in range(CH // 128):
                        nc.tensor.transpose(xp[:, 128 * tt:128 * (tt + 1)],
                                            xns[tt][:, 128 * dc:128 * (dc + 1)], ident)
                    nc.any.tensor_copy(xts[dc][:, :], xp[:, :])

                cos_c = chunks.tile([128, CH], F32, name=f"cos{n}", tag="cosc")
                sin_c = chunks.tile([128, CH], F32, name=f"sin{n}", tag="sinc")
                nc.sync.dma_start(out=cos_c, in_=cos_d[:, tcol:tcol + CH])
                nc.sync.dma_start(out=sin_c, in_=sin_d[:, tcol:tcol + CH])

                for name, wt, dst in (("q", wq, qtr), ("k", wk, ktr), ("v", wv, None)):
                    pp = pools["proj"].tile([128, CH], F32, name=f"{name}ps{n}", tag="proj")
                    for dc in range(D // 128):
                        nc.tensor.matmul(pp[:, :], wt[:, dc, :], xts[dc][:, :],
                                         start=(dc == 0), stop=(dc == D // 128 - 1))
                    if name == "v":
                        # per-head natural V via PE transpose; ones row becomes
                        # the denominator column after transpose
                        for h in range(HPC):
                            vth = chunks.tile([HD + 1, CH], F32, name=f"vt{n}{h}", tag="vth")
                            nc.scalar.activation(vth[0:HD, :], pp[HD * h:HD * (h + 1), :],
                                                 AF.Identity, bias=(bv0 if h == 0 else bv1))
                            nc.vector.memset(vth[HD:HD + 1, :], 1.0)
                            for ktl in range(CH // 128):
                                vp = pools["tp"].tile([128, HD + 1], F32, name=f"vp{n}{h}{ktl}", tag="tp")
                                nc.tensor.transpose(vp[:, :], vth[:, 128 * ktl:128 * (ktl + 1)],
                                                    ident[0:HD + 1, 0:HD + 1])
                                kt_glob = (tcol + 128 * ktl) // 128
                                nc.any.tensor_copy(
                                    vnat[kt_glob][:, 65 * h:65 * h + HD + 1], vp[:, :])
                    else:
                        # bias + rope: dst_chunk = (p+b)*cos + swap(p+b)*sin_signed
                        qb = chunks.tile([128, CH], F32R, name=f"{name}b{n}", tag="qb")
                        nc.scalar.activation(qb[:, :], pp[:, :], AF.Identity,
                                             bias=(bq if name == "q" else bk))
                        sw = pools["tp"].tile([128, CH], F32, name=f"{name}sw{n}", tag="tp")
                        for j in range(CH // 512):
                            nc.tensor.matmul(sw[:, 512 * j:512 * (j + 1)], psw,
                                             qb[:, 512 * j:512 * (j + 1)],
                                             start=True, stop=True)
                        t1 = chunks.tile([128, CH], F32, name=f"{name}t1{n}", tag="t1")
                        t2 = chunks.tile([128, CH], F32, name=f"{name}t2{n}", tag="t2")
                        nc.vector.tensor_mul(t1[:, :], qb[:, :], cos_c[:, :])
                        nc.vector.tensor_mul(t2[:, :], sw[:, :], sin_c[:, :])
                        nc.vector.tensor_add(dst[:, tcol:tcol + CH], t1[:, :], t2[:, :])

            # ── phase 3: attention for one (batch, q-chunk), both heads
            # packed: the two heads' K=64 score matmuls run concurrently in
            # disjoint PE row-groups via tile_position, so all 128 array rows
            # stay active (HAM) and scores cost one matmul-time per pair.
            def emit_att2(b, qc):
                toff = b * S
                qcols = slice(toff + QW * qc, toff + QW * (qc + 1))
                ots = [pools["ot"].tile([HD + 1, QW], F32, name=f"ot{b}{h}{qc}", tag="ot")
                       for h in range(HPC)]

                def emit_scores(kt):
                    krows = slice(toff + 128 * kt, toff + 128 * (kt + 1))
                    # both heads' scores side by side in one 2-bank psum tile;
                    # one exp instruction then covers both heads
                    sc = pools["sc"].tile([128, 2 * QW], F32, name=f"sc{b}{qc}{kt}", tag="sc")
                    for h in range(HPC):
                        po = HD * h
                        nc.tensor.matmul(sc[:, QW * h:QW * (h + 1)],
                                         ktr[po:po + HD, krows],
                                         qtr[po:po + HD, qcols], start=True, stop=True,
                                         tile_position=(po, 0))
                    if use_mask:
                        mtile = expp.tile([128, QW], F32, name=f"mt{b}{qc}{kt}", tag="mt")
                        nc.sync.dma_start(
                            out=mtile,
                            in_=mt_d[128 * kt:128 * (kt + 1), QW * qc:QW * (qc + 1)])
                        for h in range(HPC):
                            nc.vector.tensor_scalar_mul(
                                sc[:, QW * h:QW * (h + 1)], sc[:, QW * h:QW * (h + 1)], SCALE)
                            nc.vector.tensor_add(
                                sc[:, QW * h:QW * (h + 1)], sc[:, QW * h:QW * (h + 1)],
                                mtile[:, :])
                    ex = expp.tile([128, 2 * QW], BF16, name=f"ex{b}{qc}{kt}", tag="ex")
                    nc.scalar.activation(ex[:, :], sc[:, :], AF.Exp,
                                         scale=(1.0 if use_mask else SCALE))
                    return ex

                def emit_av(kt, ex):
                    kt_glob = (toff + 128 * kt) // 128
                    for h in range(HPC):
                        nc.tensor.matmul(ots[h][:, :],
                                         vnat[kt_glob][:, 65 * h:65 * h + HD + 1],
                                         ex[:, QW * h:QW * (h + 1)],
                                         start=(kt == 0), stop=(kt == KT - 1))

                # software-pipelined: scores(kt+1) issue before AV(kt)
                exs = emit_scores(0)
                for kt in range(1, KT):
                    ex_next = emit_scores(kt)
                    emit_av(kt - 1, exs)
                    exs = ex_next
                emit_av(KT - 1, exs)
                # normalize: rows 0..63 divided by row 64
                for h in range(HPC):
                    po = HD * h
                    den = outp.tile([1, QW], F32, name=f"den{b}{h}{qc}", tag="den")
                    nc.vector.tensor_copy(den[:, :], ots[h][HD:HD + 1, :])
                    bcast = outp.tile([HD, QW], F32, name=f"bc{b}{h}{qc}", tag="bc")
                    nc.gpsimd.partition_broadcast(bcast[:, :], den[:, :])
                    rcp = outp.tile([HD, QW], F32, name=f"rcp{b}{h}{qc}", tag="rcp")
                    nc.vector.reciprocal_approx_fast(rcp[:, :], bcast[:, :])
                    nc.vector.tensor_mul(aoT[po:po + HD, qcols], ots[h][0:HD, :], rcp[:, :])

            # ── phase 4: partial output projection for one (batch,qchunk)
            def emit_oproj(b, qc):
                toff = b * S
                qcols = slice(toff + QW * qc, toff + QW * (qc + 1))
                for mo in range(D // 128):
                    yp = pools["proj"].tile([128, QW], F32, name=f"yp{b}{qc}{mo}", tag="sc")
                    nc.tensor.matmul(yp[:, :], wo[:, mo, :], aoT[:, qcols],
                                     start=True, stop=True)
                    yo = outp.tile([128, QW], F32, name=f"yo{b}{qc}{mo}", tag="yo")
                    nc.vector.tensor_copy(yo[:, :], yp[:, :])
                    nc.sync.dma_start(out=ypT_d[128 * mo:128 * (mo + 1), qcols], in_=yo)

            # ── emission: sequential phases, phase-scoped psum pools ────
            ps1 = ExitStack()
            ps_tp = ps1.enter_context(tc.tile_pool(name="ps_tp", bufs=4, space="PSUM"))
            ps_proj = ps1.enter_context(tc.tile_pool(name="ps_proj", bufs=3, space="PSUM"))
            pools = {"tp": ps_tp, "proj": ps_proj}
            for n in range(NCHUNK):
                emit_chunk(n)
            ps1.close()
            ps3 = ExitStack()
            pools["sc"] = ps3.enter_context(tc.tile_pool(name="ps_sc", bufs=3, space="PSUM"))
            pools["ot"] = ps3.enter_context(tc.tile_pool(name="ps_ot", bufs=2, space="PSUM"))
            pools["proj"] = pools["sc"]
            # O-proj lags one q-chunk behind attention so the PE never waits
            # on the normalize chain at group boundaries
            groups = [(b, qc) for b in range(B) for qc in range(QC)]
            prev = None
            for g in groups:
                emit_att2(*g)
                if prev is not None:
                    emit_oproj(*prev)
                prev = g
            emit_oproj(*prev)
            ps3.close()

    nc.compile()
    nc.m = get_hw_module(nc.m)
    return nc


def _get_nc(use_mask: bool):
    key = ("nc", use_mask)
    if key not in _CACHE:
        _CACHE[key] = _build(use_mask)
    return _CACHE[key]


def kernel(x, rope, mask, Wq, bq, Wk, bk, Wv, bv, Wo, bo, _trace=False):
    x = np.ascontiguousarray(np.asarray(x, dtype=np.float32))
    rope = np.asarray(rope, dtype=np.float32)
    mask = np.asarray(mask, dtype=np.float32)
    use_mask = bool(np.any(mask))

    x2d = x.reshape(T, D)
    cos = rope[0, 0, :, 0, :]                      # [S, 64]
    sin = rope[1, 0, :, 0, :]
    sgn = np.where(np.arange(HD) % 2 == 0, -1.0, 1.0).astype(np.float32)[:, None]
    cosT = np.ascontiguousarray(np.tile(cos.T, (HPC, B)))          # [128, T]
    sinT = np.ascontiguousarray(np.tile(sin.T * sgn, (HPC, B)))    # [128, T]
    psw = np.zeros((128, 128), dtype=np.float32)
    idx = np.arange(128)
    psw[idx ^ 1, idx] = 1.0

    nc = _get_nc(use_mask)
    in_maps = []
    for c in range(NC):
        cs = slice(CW * c, CW * (c + 1))
        m = dict(
            x=x2d,
            wq=np.ascontiguousarray(Wq[:, cs]), bq=np.ascontiguousarray(bq[cs]).reshape(CW, 1),
            wk=np.ascontiguousarray(Wk[:, cs]), bk=np.ascontiguousarray(bk[cs]).reshape(CW, 1),
            wv=np.ascontiguousarray(Wv[:, cs]), bv=np.ascontiguousarray(bv[cs]).reshape(CW, 1),
            wo=np.ascontiguousarray(Wo[cs, :]),
            cost=cosT, sint=sinT, pswap=psw,
        )
        if use_mask:
            m["maskt"] = np.ascontiguousarray(mask[0, 0].T)
        in_maps.append({k: np.asarray(v, dtype=np.float32) for k, v in m.items()})

    # transient device wedges (NRT_EXEC_UNIT_UNRECOVERABLE) clear on retry
    last_err = None
    for _attempt in range(3):
        try:
            res = bass_utils.run_bass_kernel_spmd(
                nc, in_maps, core_ids=list(range(NC)), trace=_trace)
            break
        except Exception as e:  # noqa: BLE001
            last_err = e
            import time as _time
            _time.sleep(2.0)
    else:
        raise last_err
    # row-parallel unshard: sum the per-core partial projections, add bias
    ypT = res.results[0]["ypT"].astype(np.float32)
    for c in range(1, NC):
        ypT = ypT + res.results[c]["ypT"]
    out = (ypT.T + np.asarray(bo, dtype=np.float32)).reshape(B, S, D).astype(np.float32)
    out = np.ascontiguousarray(out)
    if _trace:
        return out, res
    return out


# revision 26
# speedup vs baseline: 1.4257x; 1.0374x over previous
"""Trainium2 Bass kernel for CosyVoice3 DiT attention (B=2, S=2048, H=16, hd=64, D=1024).

Sharding: tensor parallelism over heads — 2 heads per core on 8 cores.
Each core computes QKV projections for its head slice, RoPE, full attention
for its 2 heads, then its heads' contribution to the output projection
(row-parallel). The host gather sums the 8 partial outputs (the standard
row-parallel TP reduction) and adds the output bias.

Layout trick: everything is computed transposed ([dim, tokens]) so the
attention matmuls need no on-chip transposes of the big S x S matrices:
  scoresT[k,q] = K @ Q^T    (lhsT = K^T slice, rhs = Q^T slice)
  outT[d,q]    = V_aug^T @ expT  with V_aug = [V | ones] giving the softmax
                 denominator for free in row 64.
Softmax skips max-subtraction (scores are O(10) for this model family, and
exp is computed in fp32 which is safe up to ~88).

The emission order interleaves batch-0 attention with the tail of the
QKV-projection phase so the PE never idles long enough for the HAM clock
gate to re-throttle it to 1.2 GHz.
"""
import sys
sys.path.insert(0, "/opt/trn_rl_repo")
from contextlib import ExitStack
import numpy as np

# NTFF profile hook shim: this image's antenv lacks axon_hooks, which
# bass_utils imports unconditionally when trace=True (and the boot-time
# installer degrades silently without it). Recreate the module and install
# the ctypes-based hook so neuron-profile traces work.
import types as _types
try:
    import antenv as _antenv
    if "antenv.axon_hooks" not in sys.modules:
        _hooks = _types.ModuleType("antenv.axon_hooks")
        _hook_box = [None]
        _hooks.set_axon_ntff_profile_hook = lambda h: _hook_box.__setitem__(0, h)
        _hooks.get_axon_ntff_profile_hook = lambda: _hook_box[0]
        sys.modules["antenv.axon_hooks"] = _hooks
        _antenv.axon_hooks = _hooks
        try:
            from trn_agent_boot.trn_boot import _ntff_profile_via_ctypes
            _hooks.set_axon_ntff_profile_hook(
                _ntff_profile_via_ctypes("/opt/axon/libaxon_pjrt.so"))
        except Exception:
            pass
except Exception:
    pass

import concourse.bass as bass
import concourse.mybir as mybir
from concourse import bacc
from concourse.tile import TileContext
from concourse.bass_interp import get_hw_module
from concourse import bass_utils
from concourse.masks import make_identity
bass_utils.upload_artifacts = lambda tmpdir: str(tmpdir)  # no S3 in container

# ── constants (hardcoded per problem spec) ────────────────────────────────
B, S, D, H, HD = 2, 2048, 1024, 16, 64
T = B * S                 # 4096 tokens
NC = 8                    # cores
HPC = H // NC             # 2 heads per core
CW = HPC * HD             # 128 rows/cols per core
SCALE = 1.0 / np.sqrt(HD)
F32 = mybir.dt.float32
F32R = mybir.dt.float32r
BF16 = mybir.dt.bfloat16
AF = mybir.ActivationFunctionType

_CACHE = {}


def _build(use_mask: bool):
    nc = bacc.Bacc("TRN2", target_bir_lowering=False, debug=False, num_devices=NC)

    # inputs (per-core slices supplied by host)
    x_d = nc.dram_tensor("x", [T, D], F32, kind="ExternalInput")
    wq_d = nc.dram_tensor("wq", [D, CW], F32R, kind="ExternalInput")
    wk_d = nc.dram_tensor("wk", [D, CW], F32R, kind="ExternalInput")
    wv_d = nc.dram_tensor("wv", [D, CW], F32R, kind="ExternalInput")
    # wo: the CW rows of Wo owned by this core's heads -> [CW, D]
    wo_d = nc.dram_tensor("wo", [CW, D], F32R, kind="ExternalInput")
    bq_d = nc.dram_tensor("bq", [CW, 1], F32, kind="ExternalInput")
    bk_d = nc.dram_tensor("bk", [CW, 1], F32, kind="ExternalInput")
    bv_d = nc.dram_tensor("bv", [CW, 1], F32, kind="ExternalInput")
    cos_d = nc.dram_tensor("cost", [CW, T], F32, kind="ExternalInput")
    sin_d = nc.dram_tensor("sint", [CW, T], F32, kind="ExternalInput")   # sign-folded
    psw_d = nc.dram_tensor("pswap", [128, 128], F32R, kind="ExternalInput")
    if use_mask:
        mt_d = nc.dram_tensor("maskt", [S, S], F32, kind="ExternalInput")

    # partial output, transposed: ypT[n, t] = sum over this core's head dims
    ypT_d = nc.dram_tensor("ypT", [D, T], F32, kind="ExternalOutput")

    NCHUNK = 8            # token chunks of 512 for projections
    CH = T // NCHUNK      # 512
    KT = S // 128         # 16 k-tiles per batch
    QW = 512              # q chunk width
    QC = S // QW          # 4 q chunks per batch

    with TileContext(nc) as tc:
        with tc.tile_pool(name="persist", bufs=1) as persist, \
             tc.tile_pool(name="wpool", bufs=1) as wpool, \
             tc.tile_pool(name="xload", bufs=8) as xload, \
             tc.tile_pool(name="xtp", bufs=16) as xtpool, \
             tc.tile_pool(name="chunks", bufs=3) as chunks, \
             tc.tile_pool(name="expp", bufs=6) as expp, \
             tc.tile_pool(name="outp", bufs=4) as outp:

            # ── persistent tiles ────────────────────────────────────────
            ident = persist.tile([128, 128], F32, name="ident")
            make_identity(nc, ident)
            psw = persist.tile([128, 128], F32R, name="psw")
            nc.sync.dma_start(out=psw, in_=psw_d[:, :])
            wq = wpool.tile([128, D // 128, CW], F32R, name="wq_sb")
            wk = wpool.tile([128, D // 128, CW], F32R, name="wk_sb")
            wv = wpool.tile([128, D // 128, CW], F32R, name="wv_sb")
            for wt, wdr in ((wq, wq_d), (wk, wk_d), (wv, wv_d)):
                nc.sync.dma_start(out=wt, in_=wdr.ap().rearrange("(kc p) m -> p kc m", p=128))
            # wo rows for this core: [CW, D] -> lhsT chunks [CW, 128] per out-col group
            wo = wpool.tile([CW, D // 128, 128], F32R, name="wo_sb")
            nc.sync.dma_start(out=wo, in_=wo_d.ap().rearrange("p (mc m) -> p mc m", m=128))
            bq = wpool.tile([CW, 1], F32, name="bq_sb")
            bk = wpool.tile([CW, 1], F32, name="bk_sb")
            bv0 = wpool.tile([HD, 1], F32, name="bv0_sb")
            bv1 = wpool.tile([HD, 1], F32, name="bv1_sb")
            nc.sync.dma_start(out=bq, in_=bq_d[:, :])
            nc.sync.dma_start(out=bk, in_=bk_d[:, :])
            nc.sync.dma_start(out=bv0, in_=bv_d[0:HD, :])
            nc.sync.dma_start(out=bv1, in_=bv_d[HD:CW, :])

            qtr = persist.tile([128, T], BF16, name="qtr")    # rope'd Q^T
            ktr = persist.tile([128, T], BF16, name="ktr")    # rope'd K^T
            aoT = persist.tile([128, T], F32R, name="aoT")    # normalized attn out^T
            # V natural per k-tile: [128 tok, 2*(64+1)] with ones cols
            vnat = [persist.tile([128, 2 * (HD + 1)], BF16, name=f"vnat{i}")
                    for i in range(T // 128)]

            # ── phase 1: per token-chunk: transpose x, QKV proj, rope ───
            def emit_chunk(n):
                tcol = n * CH
                xts = [xtpool.tile([128, CH], F32R, name=f"xt{n}_{dc}", tag="xt")
                       for dc in range(D // 128)]
                xns = []
                for tt in range(CH // 128):
                    xn = xload.tile([128, D], F32, name=f"xn{n}_{tt}", tag="xn")
                    nc.sync.dma_start(out=xn, in_=x_d[tcol + 128 * tt: tcol + 128 * (tt + 1), :])
                    xns.append(xn)
                for dc in range(D // 128):
                    # pack the 4 token-block transposes of one d-block into one
                    # psum bank, one copy out (alternating DVE/ACT so neither
                    # engine paces the PE)
                    xp = pools["tp"].tile([128, CH], F32, name=f"xp{n}{dc}", tag="tp")
                    for tt in range(CH // 128):
                        nc.tensor.transpose(xp[:, 128 * tt:128 * (tt + 1)],
                                            xns[tt][:, 128 * dc:128 * (dc + 1)], ident)
                    if dc % 2 == 0:
                        nc.vector.tensor_copy(xts[dc][:, :], xp[:, :])
                    else:
                        nc.scalar.copy(xts[dc][:, :], xp[:, :])

                cos_c = chunks.tile([128, CH], F32, name=f"cos{n}", tag="cosc")
                sin_c = chunks.tile([128, CH], F32, name=f"sin{n}", tag="sinc")
                nc.sync.dma_start(out=cos_c, in_=cos_d[:, tcol:tcol + CH])
                nc.sync.dma_start(out=sin_c, in_=sin_d[:, tcol:tcol + CH])

                for name, wt, dst in (("q", wq, qtr), ("k", wk, ktr), ("v", wv, None)):
                    pp = pools["proj"].tile([128, CH], F32, name=f"{name}ps{n}", tag="proj")
                    for dc in range(D // 128):
                        nc.tensor.matmul(pp[:, :], wt[:, dc, :], xts[dc][:, :],
                                         start=(dc == 0), stop=(dc == D // 128 - 1))
                    if name == "v":
                        # per-head natural V via PE transpose; ones row becomes
                        # the denominator column after transpose
                        for h in range(HPC):
                            vth = chunks.tile([HD + 1, CH], F32, name=f"vt{n}{h}", tag="vth")
                            nc.scalar.activation(vth[0:HD, :], pp[HD * h:HD * (h + 1), :],
                                                 AF.Identity, bias=(bv0 if h == 0 else bv1))
                            nc.vector.memset(vth[HD:HD + 1, :], 1.0)
                            for ktl in range(CH // 128):
                                vp = pools["tp"].tile([128, HD + 1], F32, name=f"vp{n}{h}{ktl}", tag="tp")
                                nc.tensor.transpose(vp[:, :], vth[:, 128 * ktl:128 * (ktl + 1)],
                                                    ident[0:HD + 1, 0:HD + 1])
                                kt_glob = (tcol + 128 * ktl) // 128
                                if ktl % 2 == 0:
                                    nc.vector.tensor_copy(
                                        vnat[kt_glob][:, 65 * h:65 * h + HD + 1], vp[:, :])
                                else:
                                    nc.scalar.copy(
                                        vnat[kt_glob][:, 65 * h:65 * h + HD + 1], vp[:, :])
                    else:
                        # bias + rope: dst_chunk = (p+b)*cos + swap(p+b)*sin_signed
                        qb = chunks.tile([128, CH], F32R, name=f"{name}b{n}", tag="qb")
                        nc.scalar.activation(qb[:, :], pp[:, :], AF.Identity,
                                             bias=(bq if name == "q" else bk))
                        sw = pools["tp"].tile([128, CH], F32, name=f"{name}sw{n}", tag="tp")
                        for j in range(CH // 512):
                            nc.tensor.matmul(sw[:, 512 * j:512 * (j + 1)], psw,
                                             qb[:, 512 * j:512 * (j + 1)],
                                             start=True, stop=True)
                        t1 = chunks.tile([128, CH], F32, name=f"{name}t1{n}", tag="t1")
                        t2 = chunks.tile([128, CH], F32, name=f"{name}t2{n}", tag="t2")
                        nc.vector.tensor_mul(t1[:, :], qb[:, :], cos_c[:, :])
                        nc.vector.tensor_mul(t2[:, :], sw[:, :], sin_c[:, :])
                        nc.vector.tensor_add(dst[:, tcol:tcol + CH], t1[:, :], t2[:, :])

            # ── phase 3: attention for one (batch, q-chunk), both heads
            # packed: the two heads' K=64 score matmuls run concurrently in
            # disjoint PE row-groups via tile_position, so all 128 array rows
            # stay active (HAM) and scores cost one matmul-time per pair.
            def emit_att2(b, qc):
                toff = b * S
                qcols = slice(toff + QW * qc, toff + QW * (qc + 1))
                ots = [pools["ot"].tile([HD + 1, QW], F32, name=f"ot{b}{h}{qc}", tag="ot")
                       for h in range(HPC)]

                def emit_scores(kt):
                    krows = slice(toff + 128 * kt, toff + 128 * (kt + 1))
                    # both heads' scores side by side in one 2-bank psum tile;
                    # one exp instruction then covers both heads
                    sc = pools["sc"].tile([128, 2 * QW], F32, name=f"sc{b}{qc}{kt}", tag="sc")
                    for h in range(HPC):
                        po = HD * h
                        nc.tensor.matmul(sc[:, QW * h:QW * (h + 1)],
                                         ktr[po:po + HD, krows],
                                         qtr[po:po + HD, qcols], start=True, stop=True,
                                         tile_position=(po, 0))
                    if use_mask:
                        mtile = expp.tile([128, QW], F32, name=f"mt{b}{qc}{kt}", tag="mt")
                        nc.sync.dma_start(
                            out=mtile,
                            in_=mt_d[128 * kt:128 * (kt + 1), QW * qc:QW * (qc + 1)])
                        for h in range(HPC):
                            nc.vector.tensor_scalar_mul(
                                sc[:, QW * h:QW * (h + 1)], sc[:, QW * h:QW * (h + 1)], SCALE)
                            nc.vector.tensor_add(
                                sc[:, QW * h:QW * (h + 1)], sc[:, QW * h:QW * (h + 1)],
                                mtile[:, :])
                    ex = expp.tile([128, 2 * QW], BF16, name=f"ex{b}{qc}{kt}", tag="ex")
                    nc.scalar.activation(ex[:, :], sc[:, :], AF.Exp,
                                         scale=(1.0 if use_mask else SCALE))
                    return ex

                def emit_av(kt, ex):
                    kt_glob = (toff + 128 * kt) // 128
                    for h in range(HPC):
                        nc.tensor.matmul(ots[h][:, :],
                                         vnat[kt_glob][:, 65 * h:65 * h + HD + 1],
                                         ex[:, QW * h:QW * (h + 1)],
                                         start=(kt == 0), stop=(kt == KT - 1))

                # software-pipelined: scores(kt+1) issue before AV(kt)
                exs = emit_scores(0)
                for kt in range(1, KT):
                    ex_next = emit_scores(kt)
                    emit_av(kt - 1, exs)
                    exs = ex_next
                emit_av(KT - 1, exs)
                # normalize: rows 0..63 divided by row 64
                for h in range(HPC):
                    po = HD * h
                    den = outp.tile([1, QW], F32, name=f"den{b}{h}{qc}", tag="den")
                    nc.vector.tensor_copy(den[:, :], ots[h][HD:HD + 1, :])
                    bcast = outp.tile([HD, QW], F32, name=f"bc{b}{h}{qc}", tag="bc")
                    nc.gpsimd.partition_broadcast(bcast[:, :], den[:, :])
                    rcp = outp.tile([HD, QW], F32, name=f"rcp{b}{h}{qc}", tag="rcp")
                    nc.vector.reciprocal_approx_fast(rcp[:, :], bcast[:, :])
                    nc.vector.tensor_mul(aoT[po:po + HD, qcols], ots[h][0:HD, :], rcp[:, :])

            # ── phase 4: partial output projection for one (batch,qchunk)
            def emit_oproj(b, qc):
                toff = b * S
                qcols = slice(toff + QW * qc, toff + QW * (qc + 1))
                for mo in range(D // 128):
                    yp = pools["proj"].tile([128, QW], F32, name=f"yp{b}{qc}{mo}", tag="sc")
                    nc.tensor.matmul(yp[:, :], wo[:, mo, :], aoT[:, qcols],
                                     start=True, stop=True)
                    yo = outp.tile([128, QW], F32, name=f"yo{b}{qc}{mo}", tag="yo")
                    nc.vector.tensor_copy(yo[:, :], yp[:, :])
                    nc.sync.dma_start(out=ypT_d[128 * mo:128 * (mo + 1), qcols], in_=yo)

            # ── emission: sequential phases, phase-scoped psum pools ────
            ps1 = ExitStack()
            ps_tp = ps1.enter_context(tc.tile_pool(name="ps_tp", bufs=5, space="PSUM"))
            ps_proj = ps1.enter_context(tc.tile_pool(name="ps_proj", bufs=3, space="PSUM"))
            pools = {"tp": ps_tp, "proj": ps_proj}
            for n in range(NCHUNK):
                emit_chunk(n)
            ps1.close()
            ps3 = ExitStack()
            pools["sc"] = ps3.enter_context(tc.tile_pool(name="ps_sc", bufs=3, space="PSUM"))
            pools["ot"] = ps3.enter_context(tc.tile_pool(name="ps_ot", bufs=2, space="PSUM"))
            pools["proj"] = pools["sc"]
            # O-proj lags one q-chunk behind attention so the PE never waits
            # on the normalize chain at group boundaries
            groups = [(b, qc) for b in range(B) for qc in range(QC)]
            prev = None
            for g in groups:
                emit_att2(*g)
                if prev is not None:
                    emit_oproj(*prev)
                prev = g
            emit_oproj(*prev)
            ps3.close()

    nc.compile()
    nc.m = get_hw_module(nc.m)
    return nc


def _get_nc(use_mask: bool):
    key = ("nc", use_mask)
    if key not in _CACHE:
        _CACHE[key] = _build(use_mask)
    return _CACHE[key]


def kernel(x, rope, mask, Wq, bq, Wk, bk, Wv, bv, Wo, bo, _trace=False):
    x = np.ascontiguousarray(np.asarray(x, dtype=np.float32))
    rope = np.asarray(rope, dtype=np.float32)
    mask = np.asarray(mask, dtype=np.float32)
    use_mask = bool(np.any(mask))

    x2d = x.reshape(T, D)
    cos = rope[0, 0, :, 0, :]                      # [S, 64]
    sin = rope[1, 0, :, 0, :]
    sgn = np.where(np.arange(HD) % 2 == 0, -1.0, 1.0).astype(np.float32)[:, None]
    cosT = np.ascontiguousarray(np.tile(cos.T, (HPC, B)))          # [128, T]
    sinT = np.ascontiguousarray(np.tile(sin.T * sgn, (HPC, B)))    # [128, T]
    psw = np.zeros((128, 128), dtype=np.float32)
    idx = np.arange(128)
    psw[idx ^ 1, idx] = 1.0

    nc = _get_nc(use_mask)
    in_maps = []
    for c in range(NC):
        cs = slice(CW * c, CW * (c + 1))
        m = dict(
            x=x2d,
            wq=np.ascontiguousarray(Wq[:, cs]), bq=np.ascontiguousarray(bq[cs]).reshape(CW, 1),
            wk=np.ascontiguousarray(Wk[:, cs]), bk=np.ascontiguousarray(bk[cs]).reshape(CW, 1),
            wv=np.ascontiguousarray(Wv[:, cs]), bv=np.ascontiguousarray(bv[cs]).reshape(CW, 1),
            wo=np.ascontiguousarray(Wo[cs, :]),
            cost=cosT, sint=sinT, pswap=psw,
        )
        if use_mask:
            m["maskt"] = np.ascontiguousarray(mask[0, 0].T)
        in_maps.append({k: np.asarray(v, dtype=np.float32) for k, v in m.items()})

    # transient device wedges (NRT_EXEC_UNIT_UNRECOVERABLE) clear on retry
    last_err = None
    for _attempt in range(3):
        try:
            res = bass_utils.run_bass_kernel_spmd(
                nc, in_maps, core_ids=list(range(NC)), trace=_trace)
            break
        except Exception as e:  # noqa: BLE001
            last_err = e
            import time as _time
            _time.sleep(2.0)
    else:
        raise last_err
    # row-parallel unshard: sum the per-core partial projections, add bias
    ypT = res.results[0]["ypT"].astype(np.float32)
    for c in range(1, NC):
        ypT = ypT + res.results[c]["ypT"]
    out = (ypT.T + np.asarray(bo, dtype=np.float32)).reshape(B, S, D).astype(np.float32)
    out = np.ascontiguousarray(out)
    if _trace:
        return out, res
    return out
